# revision 2
# baseline (speedup 1.0000x reference)
"""Causal self-attention (B=2, T=2048, C=1024, nh=16) on 8 TRN2 NeuronCores.

Sharding: core c -> batch b = c//4, head group g = c%4 (4 heads each).
Each core computes QKV projections for its heads, causal attention, and a
partial output projection (W_proj rows for its heads, + b_proj/4). The four
partials per batch are summed on the host.

Layouts (per core, hardcoded):
  xt   [128, 8, 2048]    x[b].T tiles:  xt[p, kt, t] = x[b, t, kt*128+p]
  wqk  [128, 8, 4, 128]  W_attn q|k cols for this core's heads
  wv   [128, 8, 256]     W_attn v cols
  bqk  [128, 4] f32      b_attn q|k (per-partition bias)
  bv   [1, 256]          b_attn v (added via K=1 matmul)
  wp   [128, 2, 1024]    W_proj rows for this core's heads
  bp4  [1, 1024]         b_proj / 4 (added via K=1 matmul)
  out  [2048, 1024] f32  partial (x[b] @ ... for this head group)

In-kernel dataflow (all matmuls bf16 with fp32 PSUM accumulation):
  qT,kT = (W.T @ x.T)      [feat, t] layout  (lhsT=W tile, rhs=xT)
  v     = (x @ Wv)         [t, feat] layout  (lhsT=xT tile, rhs=Wv)
  S^T   = k @ q.T          [j, i] layout     (lhsT=kT tile, rhs=qT)
  P^T   = exp(S^T/8), masked on diagonal tiles (mult by 0/1 mask)
  y^T,l = [v|1].T @ P^T    [d, i] layout, row 64 = l = sum_j P
  yT    = y^T * (1/l broadcast)
  out   = yT.T @ Wp (+ ones-row * b_proj/4)
"""

import os
import sys

sys.path.insert(0, "/opt/trn_rl_repo")
os.environ.setdefault("MYCRO_LOCAL_CACHE", "1")

import ml_dtypes
import numpy as np

import concourse.bass as bass
import concourse.mybir as mybir
import concourse.tile as tile
from concourse import bacc
from concourse.bass_utils import run_bass_kernel_spmd

B, T, C, NH, HS = 2, 2048, 1024, 16, 64
HPC = 4  # heads per core
N_CORES = 8
KT = C // 128  # 8 contraction tiles over C
TT = T // 128  # 16 tiles over T
IB = T // 512  # 4 i-blocks over T
F32 = mybir.dt.float32

CD = mybir.dt.bfloat16
CD_NP = ml_dtypes.bfloat16

LAST_RESULT = None
_CACHE = {}


def _emit(nc, tc, ctx, aps):
    xt, wqk, wv, bqk, bv, wp, bp4, out = (
        aps["xt"], aps["wqk"], aps["wv"], aps["bqk"], aps["bv"], aps["wp"],
        aps["bp4"], aps["out"],
    )
    Exp = mybir.ActivationFunctionType.Exp
    Identity = mybir.ActivationFunctionType.Identity

    consts = ctx.enter_context(tc.tile_pool(name="consts", bufs=1))

    # --- persistent SBUF tensors ---
    xt_s = consts.tile([128, KT, T], CD, tag="xt")
    nc.sync.dma_start(xt_s[:], xt)
    wqk_s = consts.tile([128, KT, 4, 128], CD, tag="wqk")
    nc.sync.dma_start(wqk_s[:], wqk)
    wv_s = consts.tile([128, KT, HPC * HS], CD, tag="wv")
    nc.sync.dma_start(wv_s[:], wv)
    bqk_s = consts.tile([128, 4], F32, tag="bqk")
    nc.sync.dma_start(bqk_s[:], bqk)
    bv_s = consts.tile([1, HPC * HS], CD, tag="bv")
    nc.sync.dma_start(bv_s[:], bv)
    wp_s = consts.tile([128, 2, C], CD, tag="wp")
    nc.sync.dma_start(wp_s[:], wp)
    bp4_s = consts.tile([1, C], CD, tag="bp4")
    nc.sync.dma_start(bp4_s[:], bp4)

    ones1 = consts.tile([1, 128], CD, tag="ones1")
    nc.vector.memset(ones1[:], 1.0)

    # causal mask, shifted-triangle trick: mask[j, c] = 1 if j <= c - 384.
    # diagonal tile with delta = 512*ib - 128*j needs slice [384+delta : +512].
    mask_s = consts.tile([128, 896], CD, tag="mask")
    nc.vector.memset(mask_s[:], 1.0)
    nc.gpsimd.affine_select(
        out=mask_s[:],
        in_=mask_s[:],
        compare_op=mybir.AluOpType.is_ge,
        fill=0.0,
        base=-384,
        channel_multiplier=-1,
        pattern=[[1, 896]],
    )

    qk_s = consts.tile([128, 4, T], CD, tag="qk")  # jt: 0,1 = q; 2,3 = k
    vext_s = consts.tile([128, TT, HPC * (HS + 1)], CD, tag="vext")
    vext4 = vext_s[:].rearrange("p t (h c) -> p t h c", c=HS + 1)
    nc.vector.memset(vext4[:, :, :, HS], 1.0)  # ones columns
    yt_s = consts.tile([128, 2, T], CD, tag="yt")

    # --- phase 1: qkv projections ---
    with tc.tile_pool(name="ph1_psum", bufs=2, space="PSUM") as ph1:
        for jt in range(4):
            for tb in range(IB):
                ps = ph1.tile([128, 512], F32, tag="qk")
                for kt in range(KT):
                    nc.tensor.matmul(
                        out=ps[:],
                        lhsT=wqk_s[:, kt, jt, :],
                        rhs=xt_s[:, kt, tb * 512:(tb + 1) * 512],
                        start=(kt == 0),
                        stop=(kt == KT - 1),
                    )
                nc.scalar.activation(
                    out=qk_s[:, jt, tb * 512:(tb + 1) * 512],
                    in_=ps[:],
                    func=Identity,
                    bias=bqk_s[:, jt:jt + 1],
                )
        for tt in range(TT):
            psv = ph1.tile([128, HPC * HS], F32, tag="v")
            for kt in range(KT):
                nc.tensor.matmul(
                    out=psv[:],
                    lhsT=xt_s[:, kt, tt * 128:(tt + 1) * 128],
                    rhs=wv_s[:, kt, :],
                    start=(kt == 0),
                    stop=False,
                )
            nc.tensor.matmul(
                out=psv[:], lhsT=ones1[:], rhs=bv_s[:], start=False, stop=True
            )
            nc.vector.tensor_copy(
                vext4[:, tt, :, 0:HS],
                psv[:].rearrange("p (h c) -> p h c", c=HS),
            )

    # --- phase 2+3: attention (i-block outer), then that block's projection ---
    attn_sp = ctx.enter_context(tc.tile_pool(name="attn_s", bufs=2, space="PSUM"))
    attn_yp = ctx.enter_context(tc.tile_pool(name="attn_y", bufs=2, space="PSUM"))
    pt_pool = ctx.enter_context(tc.tile_pool(name="pt", bufs=4))
    misc = ctx.enter_context(tc.tile_pool(name="misc", bufs=4))
    proj_p = ctx.enter_context(tc.tile_pool(name="proj", bufs=2, space="PSUM"))
    stage = ctx.enter_context(tc.tile_pool(name="stage", bufs=3))

    for ib in range(IB):
        isl = slice(ib * 512, (ib + 1) * 512)
        for h in range(HPC):
            jt_q = h // 2
            row = (h % 2) * 64
            q_ap = qk_s[row:row + 64, jt_q, isl]
            psy = attn_yp.tile([HS + 1, 512], F32, tag="y")
            njt = 4 * ib + 4
            for j in range(njt):
                psS = attn_sp.tile([128, 512], F32, tag="s")
                nc.tensor.matmul(
                    out=psS[:],
                    lhsT=qk_s[row:row + 64, 2 + jt_q, j * 128:(j + 1) * 128],
                    rhs=q_ap,
                    start=True,
                    stop=True,
                )
                pt = pt_pool.tile([128, 512], CD, tag="pt")
                nc.scalar.activation(out=pt[:], in_=psS[:], func=Exp, scale=0.125)
                delta = ib * 512 - j * 128
                if delta <= 0:  # diagonal tile -> mask
                    off = 384 + delta
                    nc.vector.tensor_mul(pt[:], pt[:], mask_s[:, off:off + 512])
                nc.tensor.matmul(
                    out=psy[:],
                    lhsT=vext4[:, j, h, :],
                    rhs=pt[:],
                    start=(j == 0),
                    stop=(j == njt - 1),
                )
            linv = misc.tile([1, 512], F32, tag="linv")
            nc.vector.reciprocal(linv[:], psy[HS:HS + 1, :])
            lbc = misc.tile([64, 512], F32, tag="lbc")
            nc.gpsimd.partition_broadcast(lbc[:], linv[:], channels=64)
            nc.vector.tensor_mul(
                yt_s[row:row + 64, jt_q, isl], psy[0:HS, :], lbc[:]
            )

        for tloc in range(4):
            ttp = ib * 4 + tloc
            for eb in range(2):
                psp = proj_p.tile([128, 512], F32, tag="p")
                for dt in range(2):
                    nc.tensor.matmul(
                        out=psp[:],
                        lhsT=yt_s[:, dt, ttp * 128:(ttp + 1) * 128],
                        rhs=wp_s[:, dt, eb * 512:(eb + 1) * 512],
                        start=(dt == 0),
                        stop=False,
                    )
                nc.tensor.matmul(
                    out=psp[:],
                    lhsT=ones1[:],
                    rhs=bp4_s[:, eb * 512:(eb + 1) * 512],
                    start=False,
                    stop=True,
                )
                st = stage.tile([128, 512], F32, tag="st")
                nc.vector.tensor_copy(st[:], psp[:])
                nc.sync.dma_start(
                    out[ttp * 128:(ttp + 1) * 128, eb * 512:(eb + 1) * 512], st[:]
                )


def build():
    if "nc" in _CACHE:
        return _CACHE["nc"]
    nc = bacc.Bacc(
        "TRN2", target_bir_lowering=False, debug=False, num_devices=N_CORES
    )
    aps = {
        "xt": nc.dram_tensor("xt", [128, KT, T], CD, kind="ExternalInput").ap(),
        "wqk": nc.dram_tensor("wqk", [128, KT, 4, 128], CD, kind="ExternalInput").ap(),
        "wv": nc.dram_tensor("wv", [128, KT, HPC * HS], CD, kind="ExternalInput").ap(),
        "bqk": nc.dram_tensor("bqk", [128, 4], F32, kind="ExternalInput").ap(),
        "bv": nc.dram_tensor("bv", [1, HPC * HS], CD, kind="ExternalInput").ap(),
        "wp": nc.dram_tensor("wp", [128, 2, C], CD, kind="ExternalInput").ap(),
        "bp4": nc.dram_tensor("bp4", [1, C], CD, kind="ExternalInput").ap(),
        "out": nc.dram_tensor("out", [T, C], F32, kind="ExternalOutput").ap(),
    }
    from contextlib import ExitStack

    with tile.TileContext(nc) as tc:
        with ExitStack() as ctx:
            _emit(nc, tc, ctx, aps)
    nc.compile()
    _CACHE["nc"] = nc
    return nc


def make_in_maps(x, W_attn, b_attn, W_proj, b_proj):
    x = np.asarray(x, dtype=np.float32)
    W_attn = np.asarray(W_attn, dtype=np.float32)
    b_attn = np.asarray(b_attn, dtype=np.float32)
    W_proj = np.asarray(W_proj, dtype=np.float32)
    b_proj = np.asarray(b_proj, dtype=np.float32)

    in_maps = []
    xt_b = {}
    for b in range(B):
        xt = np.ascontiguousarray(x[b].T)  # [C, T]
        xt_b[b] = (
            xt.reshape(KT, 128, T).transpose(1, 0, 2).astype(CD_NP)
        )
    for core in range(N_CORES):
        b = core // 4
        g = core % 4
        fs = slice(256 * g, 256 * g + 256)  # feature cols for this head group
        wq = W_attn[:, fs]
        wk = W_attn[:, C + 256 * g: C + 256 * g + 256]
        wv = W_attn[:, 2 * C + 256 * g: 2 * C + 256 * g + 256]
        wqk = np.concatenate([wq, wk], axis=1)  # [1024, 512]
        bq = b_attn[fs]
        bk = b_attn[C + 256 * g: C + 256 * g + 256]
        bv = b_attn[2 * C + 256 * g: 2 * C + 256 * g + 256]
        in_maps.append({
            "xt": xt_b[b],
            "wqk": np.ascontiguousarray(
                wqk.reshape(KT, 128, 4, 128).transpose(1, 0, 2, 3)
            ).astype(CD_NP),
            "wv": np.ascontiguousarray(
                wv.reshape(KT, 128, 256).transpose(1, 0, 2)
            ).astype(CD_NP),
            "bqk": np.ascontiguousarray(
                np.concatenate([bq, bk]).reshape(4, 128).T
            ).astype(np.float32),
            "bv": bv[None, :].astype(CD_NP),
            "wp": np.ascontiguousarray(
                W_proj[fs, :].reshape(2, 128, C).transpose(1, 0, 2)
            ).astype(CD_NP),
            "bp4": (b_proj / 4)[None, :].astype(CD_NP),
        })
    return in_maps


def _ensure_ntff_hook():
    """Recreate the missing antenv.axon_hooks NTFF-profile shim (see
    trn_agent_boot/trn_boot.py) so run_bass_kernel_spmd(trace=True) works."""
    import contextlib
    import ctypes
    import types

    try:
        from antenv.axon_hooks import get_axon_ntff_profile_hook  # noqa: F401

        return
    except ImportError:
        pass

    mod = types.ModuleType("antenv.axon_hooks")
    _holder = {"hook": None}
    mod.set_axon_ntff_profile_hook = lambda h: _holder.__setitem__("hook", h)
    mod.get_axon_ntff_profile_hook = lambda: _holder["hook"]
    sys.modules["antenv.axon_hooks"] = mod
    import antenv

    antenv.axon_hooks = mod

    so_path = "/opt/axon/libaxon_pjrt.so"
    if not os.path.exists(so_path):
        return
    lib = ctypes.CDLL(so_path)
    if not hasattr(lib, "axon_start_nrt_profile"):
        return
    lib.axon_start_nrt_profile.argtypes = [
        ctypes.POINTER(ctypes.c_int64),
        ctypes.c_size_t,
    ]
    lib.axon_start_nrt_profile.restype = ctypes.c_int64
    lib.axon_stop_nrt_profile.argtypes = [ctypes.c_char_p]
    lib.axon_stop_nrt_profile.restype = ctypes.c_int64

    @contextlib.contextmanager
    def _hook(output_dir, device_ids):
        import jax

        jax.devices()
        if device_ids:
            ids = (ctypes.c_int64 * len(device_ids))(*device_ids)
            rc = lib.axon_start_nrt_profile(ids, len(device_ids))
        else:
            rc = lib.axon_start_nrt_profile(None, 0)
        if rc != 0:
            raise RuntimeError(f"axon_start_nrt_profile rc={rc}")
        try:
            yield
        finally:
            n = lib.axon_stop_nrt_profile(str(output_dir).encode())
            if n <= 0:
                print(f"ntff profile: rc={n}, nothing written to {output_dir}")

    mod.set_axon_ntff_profile_hook(_hook)


def kernel(x, W_attn, b_attn, W_proj, b_proj):
    global LAST_RESULT
    nc = build()
    in_maps = make_in_maps(x, W_attn, b_attn, W_proj, b_proj)
    trace = os.environ.get("KERNEL_TRACE", "0") == "1"
    if trace:
        _ensure_ntff_hook()
        import concourse.bass_utils as _bu

        _bu.upload_artifacts = lambda tmpdir: f"local://{tmpdir}"
    res = run_bass_kernel_spmd(
        nc, in_maps, core_ids=list(range(N_CORES)), trace=trace
    )
    LAST_RESULT = res
    outs = [res.results[i]["out"] for i in range(N_CORES)]
    y = np.empty((B, T, C), dtype=np.float32)
    for b in range(B):
        y[b] = outs[4 * b] + outs[4 * b + 1] + outs[4 * b + 2] + outs[4 * b + 3]
    return y


# revision 6
# speedup vs baseline: 1.0749x; 1.0749x over previous
"""Causal self-attention (B=2, T=2048, C=1024, nh=16) on 8 TRN2 NeuronCores.

Sharding: core c -> batch b = c//4, head group g = c%4 (4 heads each).
Each core computes QKV projections for its heads, causal attention, and a
partial output projection (W_proj rows for its heads, + b_proj/4). The four
partials per batch are summed on the host.

Layouts (per core, hardcoded):
  xt   [128, 8, 2048]    x[b].T tiles:  xt[p, kt, t] = x[b, t, kt*128+p]
  wqk  [128, 8, 4, 128]  W_attn q|k cols for this core's heads
  wv   [128, 8, 256]     W_attn v cols
  bqk  [128, 4] f32      b_attn q|k (per-partition bias)
  bv   [1, 256]          b_attn v (added via K=1 matmul)
  wp   [128, 2, 1024]    W_proj rows for this core's heads
  bp4  [1, 1024]         b_proj / 4 (added via K=1 matmul)
  out  [2048, 1024] f32  partial (x[b] @ ... for this head group)

In-kernel dataflow (all matmuls bf16 with fp32 PSUM accumulation):
  qT,kT = (W.T @ x.T)      [feat, t] layout  (lhsT=W tile, rhs=xT)
  v     = (x @ Wv)         [t, feat] layout  (lhsT=xT tile, rhs=Wv)
  S^T   = k @ q.T          [j, i] layout     (lhsT=kT tile, rhs=qT)
  P^T   = exp(S^T/8), masked on diagonal tiles (mult by 0/1 mask)
  y^T,l = [v|1].T @ P^T    [d, i] layout, row 64 = l = sum_j P
  yT    = y^T * (1/l broadcast)
  out   = yT.T @ Wp (+ ones-row * b_proj/4)
"""

import os
import sys

sys.path.insert(0, "/opt/trn_rl_repo")
os.environ.setdefault("MYCRO_LOCAL_CACHE", "1")

import ml_dtypes
import numpy as np

import concourse.bass as bass
import concourse.mybir as mybir
import concourse.tile as tile
from concourse import bacc
from concourse.bass_utils import run_bass_kernel_spmd

B, T, C, NH, HS = 2, 2048, 1024, 16, 64
HPC = 4  # heads per core
N_CORES = 8
KT = C // 128  # 8 contraction tiles over C
TT = T // 128  # 16 tiles over T
IB = T // 512  # 4 i-blocks over T
F32 = mybir.dt.float32

CD = mybir.dt.bfloat16
CD_NP = ml_dtypes.bfloat16

LAST_RESULT = None
_CACHE = {}


def _emit(nc, tc, ctx, aps):
    xt, wqk, wv, bqk, bv, wp, bp4, out = (
        aps["xt"], aps["wqk"], aps["wv"], aps["bqk"], aps["bv"], aps["wp"],
        aps["bp4"], aps["out"],
    )
    Exp = mybir.ActivationFunctionType.Exp
    Identity = mybir.ActivationFunctionType.Identity

    consts = ctx.enter_context(tc.tile_pool(name="consts", bufs=1))

    # --- persistent SBUF tensors ---
    xt_s = consts.tile([128, KT, T], CD, tag="xt")
    nc.sync.dma_start(xt_s[:], xt)
    wqk_s = consts.tile([128, KT, 4, 128], CD, tag="wqk")
    nc.sync.dma_start(wqk_s[:], wqk)
    wv_s = consts.tile([128, KT, HPC * HS], CD, tag="wv")
    nc.sync.dma_start(wv_s[:], wv)
    bqk_s = consts.tile([128, 4], F32, tag="bqk")
    nc.sync.dma_start(bqk_s[:], bqk)
    bv_s = consts.tile([1, HPC * HS], CD, tag="bv")
    nc.sync.dma_start(bv_s[:], bv)
    wp_s = consts.tile([128, 2, C], CD, tag="wp")
    nc.sync.dma_start(wp_s[:], wp)
    bp4_s = consts.tile([1, C], CD, tag="bp4")
    nc.sync.dma_start(bp4_s[:], bp4)

    ones1 = consts.tile([1, 128], CD, tag="ones1")
    nc.vector.memset(ones1[:], 1.0)

    # causal mask, shifted-triangle trick: mask[j, c] = 1 if j <= c - 384.
    # diagonal tile with delta = 512*ib - 128*j needs slice [384+delta : +512].
    mask_s = consts.tile([128, 896], CD, tag="mask")
    nc.vector.memset(mask_s[:], 1.0)
    nc.gpsimd.affine_select(
        out=mask_s[:],
        in_=mask_s[:],
        compare_op=mybir.AluOpType.is_ge,
        fill=0.0,
        base=-384,
        channel_multiplier=-1,
        pattern=[[1, 896]],
    )

    qk_s = consts.tile([128, 4, T], CD, tag="qk")  # jt: 0,1 = q; 2,3 = k
    vext_s = consts.tile([128, TT, HPC * (HS + 1)], CD, tag="vext")
    vext4 = vext_s[:].rearrange("p t (h c) -> p t h c", c=HS + 1)
    nc.vector.memset(vext4[:, :, :, HS], 1.0)  # ones columns
    yt_s = consts.tile([128, 2, T], CD, tag="yt")

    # --- phase 1: qkv projections ---
    with tc.tile_pool(name="ph1_psum", bufs=2, space="PSUM") as ph1:
        for jt in range(4):
            for tb in range(IB):
                ps = ph1.tile([128, 512], F32, tag="qk")
                for kt in range(KT):
                    nc.tensor.matmul(
                        out=ps[:],
                        lhsT=wqk_s[:, kt, jt, :],
                        rhs=xt_s[:, kt, tb * 512:(tb + 1) * 512],
                        start=(kt == 0),
                        stop=(kt == KT - 1),
                    )
                nc.scalar.activation(
                    out=qk_s[:, jt, tb * 512:(tb + 1) * 512],
                    in_=ps[:],
                    func=Identity,
                    bias=bqk_s[:, jt:jt + 1],
                )
        for tt in range(TT):
            psv = ph1.tile([128, HPC * HS], F32, tag="v")
            for kt in range(KT):
                nc.tensor.matmul(
                    out=psv[:],
                    lhsT=xt_s[:, kt, tt * 128:(tt + 1) * 128],
                    rhs=wv_s[:, kt, :],
                    start=(kt == 0),
                    stop=False,
                )
            nc.tensor.matmul(
                out=psv[:], lhsT=ones1[:], rhs=bv_s[:], start=False, stop=True
            )
            nc.vector.tensor_copy(
                vext4[:, tt, :, 0:HS],
                psv[:].rearrange("p (h c) -> p h c", c=HS),
            )

    # --- phase 2+3: attention (i-block outer), then that block's projection ---
    attn_sp = ctx.enter_context(tc.tile_pool(name="attn_s", bufs=3, space="PSUM"))
    attn_yp = ctx.enter_context(tc.tile_pool(name="attn_y", bufs=2, space="PSUM"))
    pt_pool = ctx.enter_context(tc.tile_pool(name="pt", bufs=6))
    misc = ctx.enter_context(tc.tile_pool(name="misc", bufs=4))
    proj_p = ctx.enter_context(tc.tile_pool(name="proj", bufs=3, space="PSUM"))
    stage = ctx.enter_context(tc.tile_pool(name="stage", bufs=3))

    # fixed 128x128 causal triangle (j' <= c') for diagonal strips
    tri = mask_s[:, 384:512]

    for ib in range(IB):
        isl = slice(ib * 512, (ib + 1) * 512)
        for h in range(HPC):
            jt_q = h // 2
            row = (h % 2) * 64
            psy = attn_yp.tile([HS + 1, 512], F32, tag="y")
            njt = 4 * ib + 4
            for j in range(njt):
                # diagonal tiles only contribute to columns i' >= o
                o = max(0, 128 * j - 512 * ib)
                w = 512 - o
                psS = attn_sp.tile([128, 512], F32, tag="s")
                nc.tensor.matmul(
                    out=psS[:, 0:w],
                    lhsT=qk_s[row:row + 64, 2 + jt_q, j * 128:(j + 1) * 128],
                    rhs=qk_s[row:row + 64, jt_q, ib * 512 + o:(ib + 1) * 512],
                    start=True,
                    stop=True,
                )
                pt = pt_pool.tile([128, 512], CD, tag="pt")
                nc.scalar.activation(
                    out=pt[:, o:512], in_=psS[:, 0:w], func=Exp, scale=0.125
                )
                if 128 * j >= 512 * ib:  # diagonal tile -> mask boundary strip
                    nc.vector.tensor_mul(
                        pt[:, o:o + 128], pt[:, o:o + 128], tri
                    )
                nc.tensor.matmul(
                    out=psy[:, o:512],
                    lhsT=vext4[:, j, h, :],
                    rhs=pt[:, o:512],
                    start=(j == 0),
                    stop=(j == njt - 1),
                )
            lrow = misc.tile([1, 512], F32, tag="lrow")
            nc.vector.tensor_copy(lrow[:], psy[HS:HS + 1, :])
            lbc = misc.tile([64, 512], F32, tag="lbc")
            nc.gpsimd.partition_broadcast(lbc[:], lrow[:], channels=64)
            linv = misc.tile([64, 512], F32, tag="linv")
            nc.vector.reciprocal(linv[:], lbc[:])
            nc.vector.tensor_mul(
                yt_s[row:row + 64, jt_q, isl], psy[0:HS, :], linv[:]
            )

        for tloc in range(4):
            ttp = ib * 4 + tloc
            for eb in range(2):
                psp = proj_p.tile([128, 512], F32, tag="p")
                for dt in range(2):
                    nc.tensor.matmul(
                        out=psp[:],
                        lhsT=yt_s[:, dt, ttp * 128:(ttp + 1) * 128],
                        rhs=wp_s[:, dt, eb * 512:(eb + 1) * 512],
                        start=(dt == 0),
                        stop=False,
                    )
                nc.tensor.matmul(
                    out=psp[:],
                    lhsT=ones1[:],
                    rhs=bp4_s[:, eb * 512:(eb + 1) * 512],
                    start=False,
                    stop=True,
                )
                st = stage.tile([128, 512], F32, tag="st")
                nc.vector.tensor_copy(st[:], psp[:])
                nc.sync.dma_start(
                    out[ttp * 128:(ttp + 1) * 128, eb * 512:(eb + 1) * 512], st[:]
                )


def build():
    if "nc" in _CACHE:
        return _CACHE["nc"]
    nc = bacc.Bacc(
        "TRN2", target_bir_lowering=False, debug=False, num_devices=N_CORES
    )
    aps = {
        "xt": nc.dram_tensor("xt", [128, KT, T], CD, kind="ExternalInput").ap(),
        "wqk": nc.dram_tensor("wqk", [128, KT, 4, 128], CD, kind="ExternalInput").ap(),
        "wv": nc.dram_tensor("wv", [128, KT, HPC * HS], CD, kind="ExternalInput").ap(),
        "bqk": nc.dram_tensor("bqk", [128, 4], F32, kind="ExternalInput").ap(),
        "bv": nc.dram_tensor("bv", [1, HPC * HS], CD, kind="ExternalInput").ap(),
        "wp": nc.dram_tensor("wp", [128, 2, C], CD, kind="ExternalInput").ap(),
        "bp4": nc.dram_tensor("bp4", [1, C], CD, kind="ExternalInput").ap(),
        "out": nc.dram_tensor("out", [T, C], F32, kind="ExternalOutput").ap(),
    }
    from contextlib import ExitStack

    with tile.TileContext(nc) as tc:
        with ExitStack() as ctx:
            _emit(nc, tc, ctx, aps)
    nc.compile()
    _CACHE["nc"] = nc
    return nc


def make_in_maps(x, W_attn, b_attn, W_proj, b_proj):
    x = np.asarray(x, dtype=np.float32)
    W_attn = np.asarray(W_attn, dtype=np.float32)
    b_attn = np.asarray(b_attn, dtype=np.float32)
    W_proj = np.asarray(W_proj, dtype=np.float32)
    b_proj = np.asarray(b_proj, dtype=np.float32)

    in_maps = []
    xt_b = {}
    for b in range(B):
        xt = np.ascontiguousarray(x[b].T)  # [C, T]
        xt_b[b] = (
            xt.reshape(KT, 128, T).transpose(1, 0, 2).astype(CD_NP)
        )
    for core in range(N_CORES):
        b = core // 4
        g = core % 4
        fs = slice(256 * g, 256 * g + 256)  # feature cols for this head group
        wq = W_attn[:, fs]
        wk = W_attn[:, C + 256 * g: C + 256 * g + 256]
        wv = W_attn[:, 2 * C + 256 * g: 2 * C + 256 * g + 256]
        wqk = np.concatenate([wq, wk], axis=1)  # [1024, 512]
        bq = b_attn[fs]
        bk = b_attn[C + 256 * g: C + 256 * g + 256]
        bv = b_attn[2 * C + 256 * g: 2 * C + 256 * g + 256]
        in_maps.append({
            "xt": xt_b[b],
            "wqk": np.ascontiguousarray(
                wqk.reshape(KT, 128, 4, 128).transpose(1, 0, 2, 3)
            ).astype(CD_NP),
            "wv": np.ascontiguousarray(
                wv.reshape(KT, 128, 256).transpose(1, 0, 2)
            ).astype(CD_NP),
            "bqk": np.ascontiguousarray(
                np.concatenate([bq, bk]).reshape(4, 128).T
            ).astype(np.float32),
            "bv": bv[None, :].astype(CD_NP),
            "wp": np.ascontiguousarray(
                W_proj[fs, :].reshape(2, 128, C).transpose(1, 0, 2)
            ).astype(CD_NP),
            "bp4": (b_proj / 4)[None, :].astype(CD_NP),
        })
    return in_maps


def _ensure_ntff_hook():
    """Recreate the missing antenv.axon_hooks NTFF-profile shim (see
    trn_agent_boot/trn_boot.py) so run_bass_kernel_spmd(trace=True) works."""
    import contextlib
    import ctypes
    import types

    try:
        from antenv.axon_hooks import get_axon_ntff_profile_hook  # noqa: F401

        return
    except ImportError:
        pass

    mod = types.ModuleType("antenv.axon_hooks")
    _holder = {"hook": None}
    mod.set_axon_ntff_profile_hook = lambda h: _holder.__setitem__("hook", h)
    mod.get_axon_ntff_profile_hook = lambda: _holder["hook"]
    sys.modules["antenv.axon_hooks"] = mod
    import antenv

    antenv.axon_hooks = mod

    so_path = "/opt/axon/libaxon_pjrt.so"
    if not os.path.exists(so_path):
        return
    lib = ctypes.CDLL(so_path)
    if not hasattr(lib, "axon_start_nrt_profile"):
        return
    lib.axon_start_nrt_profile.argtypes = [
        ctypes.POINTER(ctypes.c_int64),
        ctypes.c_size_t,
    ]
    lib.axon_start_nrt_profile.restype = ctypes.c_int64
    lib.axon_stop_nrt_profile.argtypes = [ctypes.c_char_p]
    lib.axon_stop_nrt_profile.restype = ctypes.c_int64

    @contextlib.contextmanager
    def _hook(output_dir, device_ids):
        import jax

        jax.devices()
        if device_ids:
            ids = (ctypes.c_int64 * len(device_ids))(*device_ids)
            rc = lib.axon_start_nrt_profile(ids, len(device_ids))
        else:
            rc = lib.axon_start_nrt_profile(None, 0)
        if rc != 0:
            raise RuntimeError(f"axon_start_nrt_profile rc={rc}")
        try:
            yield
        finally:
            n = lib.axon_stop_nrt_profile(str(output_dir).encode())
            if n <= 0:
                print(f"ntff profile: rc={n}, nothing written to {output_dir}")

    mod.set_axon_ntff_profile_hook(_hook)


def kernel(x, W_attn, b_attn, W_proj, b_proj):
    global LAST_RESULT
    nc = build()
    in_maps = make_in_maps(x, W_attn, b_attn, W_proj, b_proj)
    trace = os.environ.get("KERNEL_TRACE", "0") == "1"
    if trace:
        _ensure_ntff_hook()
        import concourse.bass_utils as _bu

        _bu.upload_artifacts = lambda tmpdir: f"local://{tmpdir}"
    res = run_bass_kernel_spmd(
        nc, in_maps, core_ids=list(range(N_CORES)), trace=trace
    )
    LAST_RESULT = res
    outs = [res.results[i]["out"] for i in range(N_CORES)]
    y = np.empty((B, T, C), dtype=np.float32)
    for b in range(B):
        y[b] = outs[4 * b] + outs[4 * b + 1] + outs[4 * b + 2] + outs[4 * b + 3]
    return y


# revision 9
# speedup vs baseline: 1.1830x; 1.1006x over previous
"""Causal self-attention (B=2, T=2048, C=1024, nh=16) on 8 TRN2 NeuronCores.

Sharding: core c -> batch b = c//4, head group g = c%4 (4 heads each).
Each core computes QKV projections for its heads, causal attention, and a
partial output projection (W_proj rows for its heads, + b_proj/4). The four
partials per batch are summed on the host.

Layouts (per core, hardcoded):
  xt   [128, 8, 2048]    x[b].T tiles:  xt[p, kt, t] = x[b, t, kt*128+p]
  wqk  [128, 8, 4, 128]  W_attn q|k cols for this core's heads
  wv   [128, 8, 256]     W_attn v cols
  bqk  [128, 4] f32      b_attn q|k (per-partition bias)
  bv   [1, 256]          b_attn v (added via K=1 matmul)
  wp   [128, 2, 1024]    W_proj rows for this core's heads
  bp4  [1, 1024]         b_proj / 4 (added via K=1 matmul)
  out  [2048, 1024] f32  partial (x[b] @ ... for this head group)

In-kernel dataflow (all matmuls bf16 with fp32 PSUM accumulation):
  qT,kT = (W.T @ x.T)      [feat, t] layout  (lhsT=W tile, rhs=xT)
  v     = (x @ Wv)         [t, feat] layout  (lhsT=xT tile, rhs=Wv)
  S^T   = k @ q.T          [j, i] layout     (lhsT=kT tile, rhs=qT)
  P^T   = exp(S^T/8), masked on diagonal tiles (mult by 0/1 mask)
  y^T,l = [v|1].T @ P^T    [d, i] layout, row 64 = l = sum_j P
  yT    = y^T * (1/l broadcast)
  out   = yT.T @ Wp (+ ones-row * b_proj/4)
"""

import os
import sys

sys.path.insert(0, "/opt/trn_rl_repo")
os.environ.setdefault("MYCRO_LOCAL_CACHE", "1")

import ml_dtypes
import numpy as np

import concourse.bass as bass
import concourse.mybir as mybir
import concourse.tile as tile
from concourse import bacc
from concourse.bass_utils import run_bass_kernel_spmd

B, T, C, NH, HS = 2, 2048, 1024, 16, 64
HPC = 4  # heads per core
N_CORES = 8
KT = C // 128  # 8 contraction tiles over C
TT = T // 128  # 16 tiles over T
IB = T // 512  # 4 i-blocks over T
F32 = mybir.dt.float32

CD = mybir.dt.bfloat16
CD_NP = ml_dtypes.bfloat16

LAST_RESULT = None
_CACHE = {}


def _emit(nc, tc, ctx, aps):
    xt, wqk, wv, bqk, bv, wp, bp4, out = (
        aps["xt"], aps["wqk"], aps["wv"], aps["bqk"], aps["bv"], aps["wp"],
        aps["bp4"], aps["out"],
    )
    Exp = mybir.ActivationFunctionType.Exp
    Identity = mybir.ActivationFunctionType.Identity

    consts = ctx.enter_context(tc.tile_pool(name="consts", bufs=1))

    # --- persistent SBUF tensors ---
    xt_s = consts.tile([128, KT, T], CD, tag="xt")
    nc.sync.dma_start(xt_s[:], xt)
    wqk_s = consts.tile([128, KT, 4, 128], CD, tag="wqk")
    nc.sync.dma_start(wqk_s[:], wqk)
    wv_s = consts.tile([128, KT, HPC * HS], CD, tag="wv")
    nc.sync.dma_start(wv_s[:], wv)
    bqk_s = consts.tile([128, 4], F32, tag="bqk")
    nc.sync.dma_start(bqk_s[:], bqk)
    bv_s = consts.tile([1, HPC * HS], CD, tag="bv")
    nc.sync.dma_start(bv_s[:], bv)
    wp_s = consts.tile([128, 2, C], CD, tag="wp")
    nc.sync.dma_start(wp_s[:], wp)
    bp4_s = consts.tile([1, C], CD, tag="bp4")
    nc.sync.dma_start(bp4_s[:], bp4)

    ones1 = consts.tile([1, 128], CD, tag="ones1")
    nc.vector.memset(ones1[:], 1.0)

    # causal mask, shifted-triangle trick: mask[j, c] = 1 if j <= c - 384.
    # diagonal tile with delta = 512*ib - 128*j needs slice [384+delta : +512].
    mask_s = consts.tile([128, 896], CD, tag="mask")
    nc.vector.memset(mask_s[:], 1.0)
    nc.gpsimd.affine_select(
        out=mask_s[:],
        in_=mask_s[:],
        compare_op=mybir.AluOpType.is_ge,
        fill=0.0,
        base=-384,
        channel_multiplier=-1,
        pattern=[[1, 896]],
    )

    qk_s = consts.tile([128, 4, T], CD, tag="qk")  # jt: 0,1 = q; 2,3 = k
    vext_s = consts.tile([128, TT, HPC * (HS + 1)], CD, tag="vext")
    vext4 = vext_s[:].rearrange("p t (h c) -> p t h c", c=HS + 1)
    nc.vector.memset(vext4[:, :, :, HS], 1.0)  # ones columns
    yt_s = consts.tile([128, 2, T], CD, tag="yt")

    # --- phase 1: qkv projections ---
    with tc.tile_pool(name="ph1_psum", bufs=2, space="PSUM") as ph1:
        for jt in range(4):
            for tb in range(IB):
                ps = ph1.tile([128, 512], F32, tag="qk")
                for kt in range(KT):
                    nc.tensor.matmul(
                        out=ps[:],
                        lhsT=wqk_s[:, kt, jt, :],
                        rhs=xt_s[:, kt, tb * 512:(tb + 1) * 512],
                        start=(kt == 0),
                        stop=(kt == KT - 1),
                    )
                nc.vector.tensor_scalar_add(
                    qk_s[:, jt, tb * 512:(tb + 1) * 512],
                    ps[:],
                    bqk_s[:, jt:jt + 1],
                )
        for tt in range(TT):
            psv = ph1.tile([128, HPC * HS], F32, tag="v")
            for kt in range(KT):
                nc.tensor.matmul(
                    out=psv[:],
                    lhsT=xt_s[:, kt, tt * 128:(tt + 1) * 128],
                    rhs=wv_s[:, kt, :],
                    start=(kt == 0),
                    stop=False,
                )
            nc.tensor.matmul(
                out=psv[:], lhsT=ones1[:], rhs=bv_s[:], start=False, stop=True
            )
            nc.vector.tensor_copy(
                vext4[:, tt, :, 0:HS],
                psv[:].rearrange("p (h c) -> p h c", c=HS),
            )

    # --- phase 2+3: attention (i-block outer), then that block's projection ---
    # S-tiles are computed two-at-a-time into one 2-bank PSUM tile so a
    # single ACTIVATE(Exp) covers 1024 columns (amortizes ~260ns/op overhead)
    attn_sp = ctx.enter_context(tc.tile_pool(name="attn_s", bufs=2, space="PSUM"))
    attn_yp = ctx.enter_context(tc.tile_pool(name="attn_y", bufs=2, space="PSUM"))
    pt_pool = ctx.enter_context(tc.tile_pool(name="pt", bufs=4))
    misc = ctx.enter_context(tc.tile_pool(name="misc", bufs=4))
    proj_p = ctx.enter_context(tc.tile_pool(name="proj", bufs=2, space="PSUM"))
    stage = ctx.enter_context(tc.tile_pool(name="stage", bufs=3))

    # fixed 128x128 causal triangle (j' <= c') for diagonal strips
    tri = mask_s[:, 384:512]

    for ib in range(IB):
        isl = slice(ib * 512, (ib + 1) * 512)
        for h in range(HPC):
            jt_q = h // 2
            row = (h % 2) * 64
            k_ap = qk_s[row:row + 64, 2 + jt_q, :]
            psy = attn_yp.tile([HS + 1, 512], F32, tag="y")
            njt = 4 * ib + 4
            # group j-tiles: pairs of full tiles, then the 4 diagonal tiles
            groups = [(j, j + 1) for j in range(0, 4 * ib, 2)]
            groups += [(j,) for j in range(4 * ib, njt)]
            for grp in groups:
                n = len(grp)
                o0 = max(0, 128 * grp[0] - 512 * ib)
                psS = attn_sp.tile([128, 1024], F32, tag="s")
                pt = pt_pool.tile([128, n, 512], CD, tag="pt")
                for gi, j in enumerate(grp):
                    o = max(0, 128 * j - 512 * ib)
                    nc.tensor.matmul(
                        out=psS[:, gi * 512: gi * 512 + (512 - o)],
                        lhsT=k_ap[:, j * 128:(j + 1) * 128],
                        rhs=qk_s[row:row + 64, jt_q, ib * 512 + o:(ib + 1) * 512],
                        start=True,
                        stop=True,
                    )
                # one exp for the whole group (diagonal tiles: o0 cols skipped)
                nc.scalar.activation(
                    out=pt[:].rearrange("p n f -> p (n f)")[:, 0: n * 512 - o0],
                    in_=psS[:, 0: n * 512 - o0],
                    func=Exp,
                    scale=0.125,
                )
                for gi, j in enumerate(grp):
                    o = max(0, 128 * j - 512 * ib)
                    if 128 * j >= 512 * ib:  # diagonal tile -> mask boundary
                        nc.vector.tensor_mul(
                            pt[:, gi, o - o:o - o + 128],
                            pt[:, gi, 0:128],
                            tri,
                        )
                    nc.tensor.matmul(
                        out=psy[:, o:512],
                        lhsT=vext4[:, j, h, :],
                        rhs=pt[:, gi, 0:512 - o],
                        start=(j == 0),
                        stop=(j == njt - 1),
                    )
            lrow = misc.tile([1, 512], F32, tag="lrow")
            nc.vector.tensor_copy(lrow[:], psy[HS:HS + 1, :])
            lbc = misc.tile([64, 512], F32, tag="lbc")
            nc.gpsimd.partition_broadcast(lbc[:], lrow[:], channels=64)
            linv = misc.tile([64, 512], F32, tag="linv")
            nc.vector.reciprocal_approx_fast(linv[:], lbc[:])
            nc.vector.tensor_mul(
                yt_s[row:row + 64, jt_q, isl], psy[0:HS, :], linv[:]
            )

        for tloc in range(4):
            ttp = ib * 4 + tloc
            for eb in range(2):
                psp = proj_p.tile([128, 512], F32, tag="p")
                for dt in range(2):
                    nc.tensor.matmul(
                        out=psp[:],
                        lhsT=yt_s[:, dt, ttp * 128:(ttp + 1) * 128],
                        rhs=wp_s[:, dt, eb * 512:(eb + 1) * 512],
                        start=(dt == 0),
                        stop=False,
                    )
                nc.tensor.matmul(
                    out=psp[:],
                    lhsT=ones1[:],
                    rhs=bp4_s[:, eb * 512:(eb + 1) * 512],
                    start=False,
                    stop=True,
                )
                st = stage.tile([128, 512], F32, tag="st")
                nc.vector.tensor_copy(st[:], psp[:])
                nc.sync.dma_start(
                    out[ttp * 128:(ttp + 1) * 128, eb * 512:(eb + 1) * 512], st[:]
                )


def build():
    if "nc" in _CACHE:
        return _CACHE["nc"]
    nc = bacc.Bacc(
        "TRN2", target_bir_lowering=False, debug=False, num_devices=N_CORES
    )
    aps = {
        "xt": nc.dram_tensor("xt", [128, KT, T], CD, kind="ExternalInput").ap(),
        "wqk": nc.dram_tensor("wqk", [128, KT, 4, 128], CD, kind="ExternalInput").ap(),
        "wv": nc.dram_tensor("wv", [128, KT, HPC * HS], CD, kind="ExternalInput").ap(),
        "bqk": nc.dram_tensor("bqk", [128, 4], F32, kind="ExternalInput").ap(),
        "bv": nc.dram_tensor("bv", [1, HPC * HS], CD, kind="ExternalInput").ap(),
        "wp": nc.dram_tensor("wp", [128, 2, C], CD, kind="ExternalInput").ap(),
        "bp4": nc.dram_tensor("bp4", [1, C], CD, kind="ExternalInput").ap(),
        "out": nc.dram_tensor("out", [T, C], F32, kind="ExternalOutput").ap(),
    }
    from contextlib import ExitStack

    with tile.TileContext(nc) as tc:
        with ExitStack() as ctx:
            _emit(nc, tc, ctx, aps)
    nc.compile()
    _CACHE["nc"] = nc
    return nc


def make_in_maps(x, W_attn, b_attn, W_proj, b_proj):
    x = np.asarray(x, dtype=np.float32)
    W_attn = np.asarray(W_attn, dtype=np.float32)
    b_attn = np.asarray(b_attn, dtype=np.float32)
    W_proj = np.asarray(W_proj, dtype=np.float32)
    b_proj = np.asarray(b_proj, dtype=np.float32)

    in_maps = []
    xt_b = {}
    for b in range(B):
        xt = np.ascontiguousarray(x[b].T)  # [C, T]
        xt_b[b] = (
            xt.reshape(KT, 128, T).transpose(1, 0, 2).astype(CD_NP)
        )
    for core in range(N_CORES):
        b = core // 4
        g = core % 4
        fs = slice(256 * g, 256 * g + 256)  # feature cols for this head group
        wq = W_attn[:, fs]
        wk = W_attn[:, C + 256 * g: C + 256 * g + 256]
        wv = W_attn[:, 2 * C + 256 * g: 2 * C + 256 * g + 256]
        wqk = np.concatenate([wq, wk], axis=1)  # [1024, 512]
        bq = b_attn[fs]
        bk = b_attn[C + 256 * g: C + 256 * g + 256]
        bv = b_attn[2 * C + 256 * g: 2 * C + 256 * g + 256]
        in_maps.append({
            "xt": xt_b[b],
            "wqk": np.ascontiguousarray(
                wqk.reshape(KT, 128, 4, 128).transpose(1, 0, 2, 3)
            ).astype(CD_NP),
            "wv": np.ascontiguousarray(
                wv.reshape(KT, 128, 256).transpose(1, 0, 2)
            ).astype(CD_NP),
            "bqk": np.ascontiguousarray(
                np.concatenate([bq, bk]).reshape(4, 128).T
            ).astype(np.float32),
            "bv": bv[None, :].astype(CD_NP),
            "wp": np.ascontiguousarray(
                W_proj[fs, :].reshape(2, 128, C).transpose(1, 0, 2)
            ).astype(CD_NP),
            "bp4": (b_proj / 4)[None, :].astype(CD_NP),
        })
    return in_maps


def _ensure_ntff_hook():
    """Recreate the missing antenv.axon_hooks NTFF-profile shim (see
    trn_agent_boot/trn_boot.py) so run_bass_kernel_spmd(trace=True) works."""
    import contextlib
    import ctypes
    import types

    try:
        from antenv.axon_hooks import get_axon_ntff_profile_hook  # noqa: F401

        return
    except ImportError:
        pass

    mod = types.ModuleType("antenv.axon_hooks")
    _holder = {"hook": None}
    mod.set_axon_ntff_profile_hook = lambda h: _holder.__setitem__("hook", h)
    mod.get_axon_ntff_profile_hook = lambda: _holder["hook"]
    sys.modules["antenv.axon_hooks"] = mod
    import antenv

    antenv.axon_hooks = mod

    so_path = "/opt/axon/libaxon_pjrt.so"
    if not os.path.exists(so_path):
        return
    lib = ctypes.CDLL(so_path)
    if not hasattr(lib, "axon_start_nrt_profile"):
        return
    lib.axon_start_nrt_profile.argtypes = [
        ctypes.POINTER(ctypes.c_int64),
        ctypes.c_size_t,
    ]
    lib.axon_start_nrt_profile.restype = ctypes.c_int64
    lib.axon_stop_nrt_profile.argtypes = [ctypes.c_char_p]
    lib.axon_stop_nrt_profile.restype = ctypes.c_int64

    @contextlib.contextmanager
    def _hook(output_dir, device_ids):
        import jax

        jax.devices()
        if device_ids:
            ids = (ctypes.c_int64 * len(device_ids))(*device_ids)
            rc = lib.axon_start_nrt_profile(ids, len(device_ids))
        else:
            rc = lib.axon_start_nrt_profile(None, 0)
        if rc != 0:
            raise RuntimeError(f"axon_start_nrt_profile rc={rc}")
        try:
            yield
        finally:
            n = lib.axon_stop_nrt_profile(str(output_dir).encode())
            if n <= 0:
                print(f"ntff profile: rc={n}, nothing written to {output_dir}")

    mod.set_axon_ntff_profile_hook(_hook)


def kernel(x, W_attn, b_attn, W_proj, b_proj):
    global LAST_RESULT
    nc = build()
    in_maps = make_in_maps(x, W_attn, b_attn, W_proj, b_proj)
    trace = os.environ.get("KERNEL_TRACE", "0") == "1"
    if trace:
        _ensure_ntff_hook()
        import concourse.bass_utils as _bu

        _bu.upload_artifacts = lambda tmpdir: f"local://{tmpdir}"
    res = run_bass_kernel_spmd(
        nc, in_maps, core_ids=list(range(N_CORES)), trace=trace
    )
    LAST_RESULT = res
    outs = [res.results[i]["out"] for i in range(N_CORES)]
    y = np.empty((B, T, C), dtype=np.float32)
    for b in range(B):
        y[b] = outs[4 * b] + outs[4 * b + 1] + outs[4 * b + 2] + outs[4 * b + 3]
    return y


# revision 13
# speedup vs baseline: 1.5657x; 1.3235x over previous
"""Causal self-attention (B=2, T=2048, C=1024, nh=16) on 8 TRN2 NeuronCores.

Sharding: core c -> batch b = c//4, head group g = c%4 (4 heads each).
Each core computes QKV projections for its heads, causal attention, and a
partial output projection (W_proj rows for its heads, + b_proj/4). The four
partials per batch are summed on the host.

Layouts (per core, hardcoded):
  xt   [128, 8, 2048]    x[b].T tiles:  xt[p, kt, t] = x[b, t, kt*128+p]
  wqk  [128, 8, 4, 128]  W_attn q|k cols for this core's heads
  wv   [128, 8, 256]     W_attn v cols
  bqk  [128, 4] f32      b_attn q|k (per-partition bias)
  bv   [1, 256]          b_attn v (added via K=1 matmul)
  wp   [128, 2, 1024]    W_proj rows for this core's heads
  bp4  [1, 1024]         b_proj / 4 (added via K=1 matmul)
  out  [2048, 1024] f32  partial (x[b] @ ... for this head group)

In-kernel dataflow (all matmuls bf16 with fp32 PSUM accumulation):
  qT,kT = (W.T @ x.T)      [feat, t] layout  (lhsT=W tile, rhs=xT)
  v     = (x @ Wv)         [t, feat] layout  (lhsT=xT tile, rhs=Wv)
  S^T   = k @ q.T          [j, i] layout     (lhsT=kT tile, rhs=qT)
  P^T   = exp(S^T/8), masked on diagonal tiles (mult by 0/1 mask)
  y^T,l = [v|1].T @ P^T    [d, i] layout, row 64 = l = sum_j P
  yT    = y^T * (1/l broadcast)
  out   = yT.T @ Wp (+ ones-row * b_proj/4)
"""

import os
import sys

sys.path.insert(0, "/opt/trn_rl_repo")
os.environ.setdefault("MYCRO_LOCAL_CACHE", "1")

import ml_dtypes
import numpy as np

import concourse.bass as bass
import concourse.mybir as mybir
import concourse.tile as tile
from concourse import bacc
from concourse.bass_utils import run_bass_kernel_spmd

B, T, C, NH, HS = 2, 2048, 1024, 16, 64
HPC = 4  # heads per core
N_CORES = 8
KT = C // 128  # 8 contraction tiles over C
TT = T // 128  # 16 tiles over T
IB = T // 512  # 4 i-blocks over T
F32 = mybir.dt.float32

CD = mybir.dt.bfloat16
CD_NP = ml_dtypes.bfloat16

LAST_RESULT = None
_CACHE = {}


def _emit(nc, tc, ctx, aps):
    xt, wqk, wv, bqk, bv, wp, bp4, out = (
        aps["xt"], aps["wqk"], aps["wv"], aps["bqk"], aps["bv"], aps["wp"],
        aps["bp4"], aps["out"],
    )
    Exp = mybir.ActivationFunctionType.Exp
    Identity = mybir.ActivationFunctionType.Identity

    consts = ctx.enter_context(tc.tile_pool(name="consts", bufs=1))

    # --- persistent SBUF tensors ---
    xt_s = consts.tile([128, KT, T], CD, tag="xt")
    nc.sync.dma_start(xt_s[:], xt)
    wqk_s = consts.tile([128, KT, 4, 128], CD, tag="wqk")
    nc.sync.dma_start(wqk_s[:], wqk)
    wv_s = consts.tile([128, KT, HPC * HS], CD, tag="wv")
    nc.sync.dma_start(wv_s[:], wv)
    bqk_s = consts.tile([128, 4], F32, tag="bqk")
    nc.sync.dma_start(bqk_s[:], bqk)
    bv_s = consts.tile([1, HPC * HS], CD, tag="bv")
    nc.sync.dma_start(bv_s[:], bv)
    wp_s = consts.tile([128, 2, C], CD, tag="wp")
    nc.sync.dma_start(wp_s[:], wp)
    bp4_s = consts.tile([1, C], CD, tag="bp4")
    nc.sync.dma_start(bp4_s[:], bp4)

    ones1 = consts.tile([1, 128], CD, tag="ones1")
    nc.vector.memset(ones1[:], 1.0)

    # causal mask, shifted-triangle trick: mask[j, c] = 1 if j <= c - 384.
    # diagonal tile with delta = 512*ib - 128*j needs slice [384+delta : +512].
    mask_s = consts.tile([128, 896], CD, tag="mask")
    nc.vector.memset(mask_s[:], 1.0)
    nc.gpsimd.affine_select(
        out=mask_s[:],
        in_=mask_s[:],
        compare_op=mybir.AluOpType.is_ge,
        fill=0.0,
        base=-384,
        channel_multiplier=-1,
        pattern=[[1, 896]],
    )

    qk_s = consts.tile([128, 2, T], CD, tag="qk")  # q only: jt 0,1
    # kT per head, zero-padded to full 128 partitions: head h occupies rows
    # (h%2)*64..+64, the other 64 rows stay zero. Full-K S-matmuls keep the
    # PE HAM clock-gate warm (K=64 row-masked MMs don't count as PE-busy).
    kz_s = consts.tile([128, HPC, T], CD, tag="kz")
    nc.vector.memset(kz_s[:], 0.0)
    vext_s = consts.tile([128, TT, HPC * (HS + 1)], CD, tag="vext")
    vext4 = vext_s[:].rearrange("p t (h c) -> p t h c", c=HS + 1)
    nc.vector.memset(vext4[:, :, :, HS], 1.0)  # ones columns
    yt_s = consts.tile([128, 2, T], CD, tag="yt")

    # --- phase 1: qkv projections ---
    with tc.tile_pool(name="ph1_psum", bufs=2, space="PSUM") as ph1:
        for jt in range(4):
            for tb in range(IB):
                ps = ph1.tile([128, 512], F32, tag="qk")
                for kt in range(KT):
                    nc.tensor.matmul(
                        out=ps[:],
                        lhsT=wqk_s[:, kt, jt, :],
                        rhs=xt_s[:, kt, tb * 512:(tb + 1) * 512],
                        start=(kt == 0),
                        stop=(kt == KT - 1),
                    )
                tsl = slice(tb * 512, (tb + 1) * 512)
                if jt < 2:  # q
                    nc.vector.tensor_scalar_add(
                        qk_s[:, jt, tsl], ps[:], bqk_s[:, jt:jt + 1]
                    )
                else:  # k -> zero-padded per-head kz
                    nc.vector.tensor_scalar_add(
                        kz_s[0:64, 2 * (jt - 2), tsl],
                        ps[0:64, :],
                        bqk_s[0:64, jt:jt + 1],
                    )
                    nc.vector.tensor_scalar_add(
                        kz_s[64:128, 2 * (jt - 2) + 1, tsl],
                        ps[64:128, :],
                        bqk_s[64:128, jt:jt + 1],
                    )
        for tt in range(TT):
            psv = ph1.tile([128, HPC * HS], F32, tag="v")
            for kt in range(KT):
                nc.tensor.matmul(
                    out=psv[:],
                    lhsT=xt_s[:, kt, tt * 128:(tt + 1) * 128],
                    rhs=wv_s[:, kt, :],
                    start=(kt == 0),
                    stop=False,
                )
            nc.tensor.matmul(
                out=psv[:], lhsT=ones1[:], rhs=bv_s[:], start=False, stop=True
            )
            nc.vector.tensor_copy(
                vext4[:, tt, :, 0:HS],
                psv[:].rearrange("p (h c) -> p h c", c=HS),
            )

    # --- phase 2+3: attention (i-block outer), then that block's projection ---
    # S-tiles are computed two-at-a-time into one 2-bank PSUM tile so a
    # single ACTIVATE(Exp) covers 1024 columns (amortizes ~260ns/op overhead)
    attn_sp = ctx.enter_context(tc.tile_pool(name="attn_s", bufs=2, space="PSUM"))
    attn_yp = ctx.enter_context(tc.tile_pool(name="attn_y", bufs=2, space="PSUM"))
    pt_pool = ctx.enter_context(tc.tile_pool(name="pt", bufs=4))
    misc = ctx.enter_context(tc.tile_pool(name="misc", bufs=4))
    proj_p = ctx.enter_context(tc.tile_pool(name="proj", bufs=2, space="PSUM"))
    stage = ctx.enter_context(tc.tile_pool(name="stage", bufs=3))

    # fixed 128x128 causal triangle (j' <= c') for diagonal strips
    tri = mask_s[:, 384:512]

    for ib in range(IB):
        isl = slice(ib * 512, (ib + 1) * 512)
        for h in range(HPC):
            jt_q = h // 2
            row = (h % 2) * 64
            psy = attn_yp.tile([HS + 1, 512], F32, tag="y")
            njt = 4 * ib + 4
            for j0 in range(0, njt, 2):
                grp = (j0, j0 + 1)
                psS = attn_sp.tile([128, 1024], F32, tag="s")
                pt = pt_pool.tile([128, 2, 512], CD, tag="pt")
                for gi, j in enumerate(grp):
                    o = max(0, 128 * j - 512 * ib)
                    nc.tensor.matmul(
                        out=psS[:, gi * 512: gi * 512 + (512 - o)],
                        lhsT=kz_s[:, h, j * 128:(j + 1) * 128],
                        rhs=qk_s[:, jt_q, ib * 512 + o:(ib + 1) * 512],
                        start=True,
                        stop=True,
                    )
                # one exp per pair when the written PSUM region is contiguous
                # (first tile full); else one exp per written segment
                o0 = max(0, 128 * j0 - 512 * ib)
                o1 = max(0, 128 * (j0 + 1) - 512 * ib)
                ptf = pt[:].rearrange("p n f -> p (n f)")
                if o0 == 0:
                    wflat = 512 + (512 - o1)
                    nc.scalar.activation(
                        out=ptf[:, 0:wflat], in_=psS[:, 0:wflat],
                        func=Exp, scale=0.125,
                    )
                else:
                    nc.scalar.activation(
                        out=ptf[:, 0:512 - o0], in_=psS[:, 0:512 - o0],
                        func=Exp, scale=0.125,
                    )
                    nc.scalar.activation(
                        out=ptf[:, 512:1024 - o1], in_=psS[:, 512:1024 - o1],
                        func=Exp, scale=0.125,
                    )
                for gi, j in enumerate(grp):
                    o = max(0, 128 * j - 512 * ib)
                    if 128 * j >= 512 * ib:  # diagonal tile -> mask boundary
                        nc.vector.tensor_mul(
                            pt[:, gi, 0:128], pt[:, gi, 0:128], tri
                        )
                    nc.tensor.matmul(
                        out=psy[:, o:512],
                        lhsT=vext4[:, j, h, :],
                        rhs=pt[:, gi, 0:512 - o],
                        start=(j == 0),
                        stop=(j == njt - 1),
                    )
            lrow = misc.tile([1, 512], F32, tag="lrow")
            nc.vector.tensor_copy(lrow[:], psy[HS:HS + 1, :])
            lbc = misc.tile([64, 512], F32, tag="lbc")
            nc.gpsimd.partition_broadcast(lbc[:], lrow[:], channels=64)
            linv = misc.tile([64, 512], F32, tag="linv")
            nc.vector.reciprocal_approx_fast(linv[:], lbc[:])
            nc.vector.tensor_mul(
                yt_s[row:row + 64, jt_q, isl], psy[0:HS, :], linv[:]
            )

        for tloc in range(4):
            ttp = ib * 4 + tloc
            for eb in range(2):
                psp = proj_p.tile([128, 512], F32, tag="p")
                for dt in range(2):
                    nc.tensor.matmul(
                        out=psp[:],
                        lhsT=yt_s[:, dt, ttp * 128:(ttp + 1) * 128],
                        rhs=wp_s[:, dt, eb * 512:(eb + 1) * 512],
                        start=(dt == 0),
                        stop=False,
                    )
                nc.tensor.matmul(
                    out=psp[:],
                    lhsT=ones1[:],
                    rhs=bp4_s[:, eb * 512:(eb + 1) * 512],
                    start=False,
                    stop=True,
                )
                st = stage.tile([128, 512], F32, tag="st")
                nc.vector.tensor_copy(st[:], psp[:])
                nc.sync.dma_start(
                    out[ttp * 128:(ttp + 1) * 128, eb * 512:(eb + 1) * 512], st[:]
                )


def build():
    if "nc" in _CACHE:
        return _CACHE["nc"]
    nc = bacc.Bacc(
        "TRN2", target_bir_lowering=False, debug=False, num_devices=N_CORES
    )
    aps = {
        "xt": nc.dram_tensor("xt", [128, KT, T], CD, kind="ExternalInput").ap(),
        "wqk": nc.dram_tensor("wqk", [128, KT, 4, 128], CD, kind="ExternalInput").ap(),
        "wv": nc.dram_tensor("wv", [128, KT, HPC * HS], CD, kind="ExternalInput").ap(),
        "bqk": nc.dram_tensor("bqk", [128, 4], F32, kind="ExternalInput").ap(),
        "bv": nc.dram_tensor("bv", [1, HPC * HS], CD, kind="ExternalInput").ap(),
        "wp": nc.dram_tensor("wp", [128, 2, C], CD, kind="ExternalInput").ap(),
        "bp4": nc.dram_tensor("bp4", [1, C], CD, kind="ExternalInput").ap(),
        "out": nc.dram_tensor("out", [T, C], F32, kind="ExternalOutput").ap(),
    }
    from contextlib import ExitStack

    with tile.TileContext(nc) as tc:
        with ExitStack() as ctx:
            _emit(nc, tc, ctx, aps)
    nc.compile()
    _CACHE["nc"] = nc
    return nc


def make_in_maps(x, W_attn, b_attn, W_proj, b_proj):
    x = np.asarray(x, dtype=np.float32)
    W_attn = np.asarray(W_attn, dtype=np.float32)
    b_attn = np.asarray(b_attn, dtype=np.float32)
    W_proj = np.asarray(W_proj, dtype=np.float32)
    b_proj = np.asarray(b_proj, dtype=np.float32)

    in_maps = []
    xt_b = {}
    for b in range(B):
        xt = np.ascontiguousarray(x[b].T)  # [C, T]
        xt_b[b] = (
            xt.reshape(KT, 128, T).transpose(1, 0, 2).astype(CD_NP)
        )
    for core in range(N_CORES):
        b = core // 4
        g = core % 4
        fs = slice(256 * g, 256 * g + 256)  # feature cols for this head group
        wq = W_attn[:, fs]
        wk = W_attn[:, C + 256 * g: C + 256 * g + 256]
        wv = W_attn[:, 2 * C + 256 * g: 2 * C + 256 * g + 256]
        wqk = np.concatenate([wq, wk], axis=1)  # [1024, 512]
        bq = b_attn[fs]
        bk = b_attn[C + 256 * g: C + 256 * g + 256]
        bv = b_attn[2 * C + 256 * g: 2 * C + 256 * g + 256]
        in_maps.append({
            "xt": xt_b[b],
            "wqk": np.ascontiguousarray(
                wqk.reshape(KT, 128, 4, 128).transpose(1, 0, 2, 3)
            ).astype(CD_NP),
            "wv": np.ascontiguousarray(
                wv.reshape(KT, 128, 256).transpose(1, 0, 2)
            ).astype(CD_NP),
            "bqk": np.ascontiguousarray(
                np.concatenate([bq, bk]).reshape(4, 128).T
            ).astype(np.float32),
            "bv": bv[None, :].astype(CD_NP),
            "wp": np.ascontiguousarray(
                W_proj[fs, :].reshape(2, 128, C).transpose(1, 0, 2)
            ).astype(CD_NP),
            "bp4": (b_proj / 4)[None, :].astype(CD_NP),
        })
    return in_maps


def _ensure_ntff_hook():
    """Recreate the missing antenv.axon_hooks NTFF-profile shim (see
    trn_agent_boot/trn_boot.py) so run_bass_kernel_spmd(trace=True) works."""
    import contextlib
    import ctypes
    import types

    try:
        from antenv.axon_hooks import get_axon_ntff_profile_hook  # noqa: F401

        return
    except ImportError:
        pass

    mod = types.ModuleType("antenv.axon_hooks")
    _holder = {"hook": None}
    mod.set_axon_ntff_profile_hook = lambda h: _holder.__setitem__("hook", h)
    mod.get_axon_ntff_profile_hook = lambda: _holder["hook"]
    sys.modules["antenv.axon_hooks"] = mod
    import antenv

    antenv.axon_hooks = mod

    so_path = "/opt/axon/libaxon_pjrt.so"
    if not os.path.exists(so_path):
        return
    lib = ctypes.CDLL(so_path)
    if not hasattr(lib, "axon_start_nrt_profile"):
        return
    lib.axon_start_nrt_profile.argtypes = [
        ctypes.POINTER(ctypes.c_int64),
        ctypes.c_size_t,
    ]
    lib.axon_start_nrt_profile.restype = ctypes.c_int64
    lib.axon_stop_nrt_profile.argtypes = [ctypes.c_char_p]
    lib.axon_stop_nrt_profile.restype = ctypes.c_int64

    @contextlib.contextmanager
    def _hook(output_dir, device_ids):
        import jax

        jax.devices()
        if device_ids:
            ids = (ctypes.c_int64 * len(device_ids))(*device_ids)
            rc = lib.axon_start_nrt_profile(ids, len(device_ids))
        else:
            rc = lib.axon_start_nrt_profile(None, 0)
        if rc != 0:
            raise RuntimeError(f"axon_start_nrt_profile rc={rc}")
        try:
            yield
        finally:
            n = lib.axon_stop_nrt_profile(str(output_dir).encode())
            if n <= 0:
                print(f"ntff profile: rc={n}, nothing written to {output_dir}")

    mod.set_axon_ntff_profile_hook(_hook)


def kernel(x, W_attn, b_attn, W_proj, b_proj):
    global LAST_RESULT
    nc = build()
    in_maps = make_in_maps(x, W_attn, b_attn, W_proj, b_proj)
    trace = os.environ.get("KERNEL_TRACE", "0") == "1"
    if trace:
        _ensure_ntff_hook()
        import concourse.bass_utils as _bu

        _bu.upload_artifacts = lambda tmpdir: f"local://{tmpdir}"
    res = run_bass_kernel_spmd(
        nc, in_maps, core_ids=list(range(N_CORES)), trace=trace
    )
    LAST_RESULT = res
    outs = [res.results[i]["out"] for i in range(N_CORES)]
    y = np.empty((B, T, C), dtype=np.float32)
    for b in range(B):
        y[b] = outs[4 * b] + outs[4 * b + 1] + outs[4 * b + 2] + outs[4 * b + 3]
    return y


# revision 16
# speedup vs baseline: 1.5706x; 1.0031x over previous
"""Causal self-attention (B=2, T=2048, C=1024, nh=16) on 8 TRN2 NeuronCores.

Sharding: core c -> batch b = c//4, head group g = c%4 (4 heads each).
Each core computes QKV projections for its heads, causal attention, and a
partial output projection (W_proj rows for its heads, + b_proj/4). The four
partials per batch are summed on the host.

Layouts (per core, hardcoded):
  xt   [128, 8, 2048]    x[b].T tiles:  xt[p, kt, t] = x[b, t, kt*128+p]
  wqk  [128, 8, 4, 128]  W_attn q|k cols for this core's heads
  wv   [128, 8, 256]     W_attn v cols
  bqk  [128, 4] f32      b_attn q|k (per-partition bias)
  bv   [1, 256]          b_attn v (added via K=1 matmul)
  wp   [128, 2, 1024]    W_proj rows for this core's heads
  bp4  [1, 1024]         b_proj / 4 (added via K=1 matmul)
  out  [2048, 1024] f32  partial (x[b] @ ... for this head group)

In-kernel dataflow (all matmuls bf16 with fp32 PSUM accumulation):
  qT,kT = (W.T @ x.T)      [feat, t] layout  (lhsT=W tile, rhs=xT)
  v     = (x @ Wv)         [t, feat] layout  (lhsT=xT tile, rhs=Wv)
  S^T   = k @ q.T          [j, i] layout     (lhsT=kT tile, rhs=qT)
  P^T   = exp(S^T/8), masked on diagonal tiles (mult by 0/1 mask)
  y^T,l = [v|1].T @ P^T    [d, i] layout, row 64 = l = sum_j P
  yT    = y^T * (1/l broadcast)
  out   = yT.T @ Wp (+ ones-row * b_proj/4)
"""

import os
import sys

sys.path.insert(0, "/opt/trn_rl_repo")
os.environ.setdefault("MYCRO_LOCAL_CACHE", "1")

import ml_dtypes
import numpy as np

import concourse.bass as bass
import concourse.mybir as mybir
import concourse.tile as tile
from concourse import bacc
from concourse.bass_utils import run_bass_kernel_spmd

B, T, C, NH, HS = 2, 2048, 1024, 16, 64
HPC = 4  # heads per core
N_CORES = 8
KT = C // 128  # 8 contraction tiles over C
TT = T // 128  # 16 tiles over T
IB = T // 512  # 4 i-blocks over T
F32 = mybir.dt.float32

CD = mybir.dt.bfloat16
CD_NP = ml_dtypes.bfloat16

LAST_RESULT = None
_CACHE = {}


def _emit(nc, tc, ctx, aps):
    xt, wqk, wv, bqk, bv, wp, bp4, out = (
        aps["xt"], aps["wqk"], aps["wv"], aps["bqk"], aps["bv"], aps["wp"],
        aps["bp4"], aps["out"],
    )
    Exp = mybir.ActivationFunctionType.Exp
    Identity = mybir.ActivationFunctionType.Identity

    consts = ctx.enter_context(tc.tile_pool(name="consts", bufs=1))

    # --- persistent SBUF tensors ---
    # per-kt tiles + DMAs: fine-grained deps let the first QKV matmuls start
    # as soon as their contraction tile lands instead of waiting for the
    # whole tensor load
    xt_k, wqk_k, wv_k = [], [], []
    for kt in range(KT):
        w = consts.tile([128, 4, 128], CD, tag=f"wqk{kt}")
        nc.sync.dma_start(w[:], wqk[:, kt])
        wqk_k.append(w)
        t = consts.tile([128, T], CD, tag=f"xt{kt}")
        nc.sync.dma_start(t[:], xt[:, kt])
        xt_k.append(t)
        v_ = consts.tile([128, HPC * HS], CD, tag=f"wv{kt}")
        nc.sync.dma_start(v_[:], wv[:, kt])
        wv_k.append(v_)
    bqk_s = consts.tile([128, 4], F32, tag="bqk")
    nc.sync.dma_start(bqk_s[:], bqk)
    bv_s = consts.tile([1, HPC * HS], CD, tag="bv")
    nc.sync.dma_start(bv_s[:], bv)
    wp_s = consts.tile([128, 2, C], CD, tag="wp")
    nc.sync.dma_start(wp_s[:], wp)
    bp4_s = consts.tile([1, C], CD, tag="bp4")
    nc.sync.dma_start(bp4_s[:], bp4)

    ones1 = consts.tile([1, 128], CD, tag="ones1")
    nc.vector.memset(ones1[:], 1.0)

    # causal mask, shifted-triangle trick: mask[j, c] = 1 if j <= c - 384.
    # diagonal tile with delta = 512*ib - 128*j needs slice [384+delta : +512].
    mask_s = consts.tile([128, 896], CD, tag="mask")
    nc.vector.memset(mask_s[:], 1.0)
    nc.gpsimd.affine_select(
        out=mask_s[:],
        in_=mask_s[:],
        compare_op=mybir.AluOpType.is_ge,
        fill=0.0,
        base=-384,
        channel_multiplier=-1,
        pattern=[[1, 896]],
    )

    qk_s = consts.tile([128, 2, T], CD, tag="qk")  # q only: jt 0,1
    # kT per head, zero-padded to full 128 partitions: head h occupies rows
    # (h%2)*64..+64, the other 64 rows stay zero. Full-K S-matmuls keep the
    # PE HAM clock-gate warm (K=64 row-masked MMs don't count as PE-busy).
    kz_s = consts.tile([128, HPC, T], CD, tag="kz")
    nc.vector.memset(kz_s[:], 0.0)
    vext_s = consts.tile([128, TT, HPC * (HS + 1)], CD, tag="vext")
    vext4 = vext_s[:].rearrange("p t (h c) -> p t h c", c=HS + 1)
    nc.vector.memset(vext4[:, :, :, HS], 1.0)  # ones columns
    yt_s = consts.tile([128, 2, T], CD, tag="yt")

    # --- phase 1: qkv projections ---
    with tc.tile_pool(name="ph1_psum", bufs=2, space="PSUM") as ph1:
        for jt in range(4):
            for tb in range(IB):
                ps = ph1.tile([128, 512], F32, tag="qk")
                for kt in range(KT):
                    nc.tensor.matmul(
                        out=ps[:],
                        lhsT=wqk_k[kt][:, jt, :],
                        rhs=xt_k[kt][:, tb * 512:(tb + 1) * 512],
                        start=(kt == 0),
                        stop=(kt == KT - 1),
                    )
                tsl = slice(tb * 512, (tb + 1) * 512)
                if jt < 2:  # q
                    nc.vector.tensor_scalar_add(
                        qk_s[:, jt, tsl], ps[:], bqk_s[:, jt:jt + 1]
                    )
                else:  # k -> zero-padded per-head kz
                    nc.vector.tensor_scalar_add(
                        kz_s[0:64, 2 * (jt - 2), tsl],
                        ps[0:64, :],
                        bqk_s[0:64, jt:jt + 1],
                    )
                    nc.vector.tensor_scalar_add(
                        kz_s[64:128, 2 * (jt - 2) + 1, tsl],
                        ps[64:128, :],
                        bqk_s[64:128, jt:jt + 1],
                    )
        for tt in range(TT):
            psv = ph1.tile([128, HPC * HS], F32, tag="v")
            for kt in range(KT):
                nc.tensor.matmul(
                    out=psv[:],
                    lhsT=xt_k[kt][:, tt * 128:(tt + 1) * 128],
                    rhs=wv_k[kt][:],
                    start=(kt == 0),
                    stop=False,
                )
            nc.tensor.matmul(
                out=psv[:], lhsT=ones1[:], rhs=bv_s[:], start=False, stop=True
            )
            nc.vector.tensor_copy(
                vext4[:, tt, :, 0:HS],
                psv[:].rearrange("p (h c) -> p h c", c=HS),
            )

    # --- phase 2+3: attention (i-block outer), then that block's projection ---
    # S-tiles are computed two-at-a-time into one 2-bank PSUM tile so a
    # single ACTIVATE(Exp) covers 1024 columns (amortizes ~260ns/op overhead)
    attn_sp = ctx.enter_context(tc.tile_pool(name="attn_s", bufs=2, space="PSUM"))
    attn_yp = ctx.enter_context(tc.tile_pool(name="attn_y", bufs=2, space="PSUM"))
    pt_pool = ctx.enter_context(tc.tile_pool(name="pt", bufs=4))
    misc = ctx.enter_context(tc.tile_pool(name="misc", bufs=4))
    proj_p = ctx.enter_context(tc.tile_pool(name="proj", bufs=2, space="PSUM"))
    stage = ctx.enter_context(tc.tile_pool(name="stage", bufs=3))

    # fixed 128x128 causal triangle (j' <= c') for diagonal strips
    tri = mask_s[:, 384:512]

    for ib in range(IB):
        isl = slice(ib * 512, (ib + 1) * 512)
        for h in range(HPC):
            jt_q = h // 2
            row = (h % 2) * 64
            psy = attn_yp.tile([HS + 1, 512], F32, tag="y")
            njt = 4 * ib + 4
            for j0 in range(0, njt, 2):
                grp = (j0, j0 + 1)
                psS = attn_sp.tile([128, 1024], F32, tag="s")
                pt = pt_pool.tile([128, 2, 512], CD, tag="pt")
                for gi, j in enumerate(grp):
                    o = max(0, 128 * j - 512 * ib)
                    nc.tensor.matmul(
                        out=psS[:, gi * 512: gi * 512 + (512 - o)],
                        lhsT=kz_s[:, h, j * 128:(j + 1) * 128],
                        rhs=qk_s[:, jt_q, ib * 512 + o:(ib + 1) * 512],
                        start=True,
                        stop=True,
                    )
                # one exp per pair when the written PSUM region is contiguous
                # (first tile full); else one exp per written segment
                o0 = max(0, 128 * j0 - 512 * ib)
                o1 = max(0, 128 * (j0 + 1) - 512 * ib)
                ptf = pt[:].rearrange("p n f -> p (n f)")
                if o0 == 0:
                    wflat = 512 + (512 - o1)
                    nc.scalar.activation(
                        out=ptf[:, 0:wflat], in_=psS[:, 0:wflat],
                        func=Exp, scale=0.125,
                    )
                else:
                    nc.scalar.activation(
                        out=ptf[:, 0:512 - o0], in_=psS[:, 0:512 - o0],
                        func=Exp, scale=0.125,
                    )
                    nc.scalar.activation(
                        out=ptf[:, 512:1024 - o1], in_=psS[:, 512:1024 - o1],
                        func=Exp, scale=0.125,
                    )
                for gi, j in enumerate(grp):
                    o = max(0, 128 * j - 512 * ib)
                    if 128 * j >= 512 * ib:  # diagonal tile -> mask boundary
                        nc.vector.tensor_mul(
                            pt[:, gi, 0:128], pt[:, gi, 0:128], tri
                        )
                    nc.tensor.matmul(
                        out=psy[:, o:512],
                        lhsT=vext4[:, j, h, :],
                        rhs=pt[:, gi, 0:512 - o],
                        start=(j == 0),
                        stop=(j == njt - 1),
                    )
            lrow = misc.tile([1, 512], F32, tag="lrow")
            nc.vector.tensor_copy(lrow[:], psy[HS:HS + 1, :])
            lbc = misc.tile([64, 512], F32, tag="lbc")
            nc.gpsimd.partition_broadcast(lbc[:], lrow[:], channels=64)
            linv = misc.tile([64, 512], F32, tag="linv")
            nc.vector.reciprocal_approx_fast(linv[:], lbc[:])
            nc.vector.tensor_mul(
                yt_s[row:row + 64, jt_q, isl], psy[0:HS, :], linv[:]
            )

        for tloc in range(4):
            ttp = ib * 4 + tloc
            for eb in range(2):
                psp = proj_p.tile([128, 512], F32, tag="p")
                for dt in range(2):
                    nc.tensor.matmul(
                        out=psp[:],
                        lhsT=yt_s[:, dt, ttp * 128:(ttp + 1) * 128],
                        rhs=wp_s[:, dt, eb * 512:(eb + 1) * 512],
                        start=(dt == 0),
                        stop=False,
                    )
                nc.tensor.matmul(
                    out=psp[:],
                    lhsT=ones1[:],
                    rhs=bp4_s[:, eb * 512:(eb + 1) * 512],
                    start=False,
                    stop=True,
                )
                st = stage.tile([128, 512], F32, tag="st")
                nc.vector.tensor_copy(st[:], psp[:])
                nc.sync.dma_start(
                    out[ttp * 128:(ttp + 1) * 128, eb * 512:(eb + 1) * 512], st[:]
                )


def build():
    if "nc" in _CACHE:
        return _CACHE["nc"]
    nc = bacc.Bacc(
        "TRN2", target_bir_lowering=False, debug=False, num_devices=N_CORES
    )
    aps = {
        "xt": nc.dram_tensor("xt", [128, KT, T], CD, kind="ExternalInput").ap(),
        "wqk": nc.dram_tensor("wqk", [128, KT, 4, 128], CD, kind="ExternalInput").ap(),
        "wv": nc.dram_tensor("wv", [128, KT, HPC * HS], CD, kind="ExternalInput").ap(),
        "bqk": nc.dram_tensor("bqk", [128, 4], F32, kind="ExternalInput").ap(),
        "bv": nc.dram_tensor("bv", [1, HPC * HS], CD, kind="ExternalInput").ap(),
        "wp": nc.dram_tensor("wp", [128, 2, C], CD, kind="ExternalInput").ap(),
        "bp4": nc.dram_tensor("bp4", [1, C], CD, kind="ExternalInput").ap(),
        "out": nc.dram_tensor("out", [T, C], F32, kind="ExternalOutput").ap(),
    }
    from contextlib import ExitStack

    with tile.TileContext(nc) as tc:
        with ExitStack() as ctx:
            _emit(nc, tc, ctx, aps)
    nc.compile()
    _CACHE["nc"] = nc
    return nc


def make_in_maps(x, W_attn, b_attn, W_proj, b_proj):
    x = np.asarray(x, dtype=np.float32)
    W_attn = np.asarray(W_attn, dtype=np.float32)
    b_attn = np.asarray(b_attn, dtype=np.float32)
    W_proj = np.asarray(W_proj, dtype=np.float32)
    b_proj = np.asarray(b_proj, dtype=np.float32)

    in_maps = []
    xt_b = {}
    for b in range(B):
        xt = np.ascontiguousarray(x[b].T)  # [C, T]
        xt_b[b] = (
            xt.reshape(KT, 128, T).transpose(1, 0, 2).astype(CD_NP)
        )
    for core in range(N_CORES):
        b = core // 4
        g = core % 4
        fs = slice(256 * g, 256 * g + 256)  # feature cols for this head group
        wq = W_attn[:, fs]
        wk = W_attn[:, C + 256 * g: C + 256 * g + 256]
        wv = W_attn[:, 2 * C + 256 * g: 2 * C + 256 * g + 256]
        wqk = np.concatenate([wq, wk], axis=1)  # [1024, 512]
        bq = b_attn[fs]
        bk = b_attn[C + 256 * g: C + 256 * g + 256]
        bv = b_attn[2 * C + 256 * g: 2 * C + 256 * g + 256]
        in_maps.append({
            "xt": xt_b[b],
            "wqk": np.ascontiguousarray(
                wqk.reshape(KT, 128, 4, 128).transpose(1, 0, 2, 3)
            ).astype(CD_NP),
            "wv": np.ascontiguousarray(
                wv.reshape(KT, 128, 256).transpose(1, 0, 2)
            ).astype(CD_NP),
            "bqk": np.ascontiguousarray(
                np.concatenate([bq, bk]).reshape(4, 128).T
            ).astype(np.float32),
            "bv": bv[None, :].astype(CD_NP),
            "wp": np.ascontiguousarray(
                W_proj[fs, :].reshape(2, 128, C).transpose(1, 0, 2)
            ).astype(CD_NP),
            "bp4": (b_proj / 4)[None, :].astype(CD_NP),
        })
    return in_maps


def _ensure_ntff_hook():
    """Recreate the missing antenv.axon_hooks NTFF-profile shim (see
    trn_agent_boot/trn_boot.py) so run_bass_kernel_spmd(trace=True) works."""
    import contextlib
    import ctypes
    import types

    try:
        from antenv.axon_hooks import get_axon_ntff_profile_hook  # noqa: F401

        return
    except ImportError:
        pass

    mod = types.ModuleType("antenv.axon_hooks")
    _holder = {"hook": None}
    mod.set_axon_ntff_profile_hook = lambda h: _holder.__setitem__("hook", h)
    mod.get_axon_ntff_profile_hook = lambda: _holder["hook"]
    sys.modules["antenv.axon_hooks"] = mod
    import antenv

    antenv.axon_hooks = mod

    so_path = "/opt/axon/libaxon_pjrt.so"
    if not os.path.exists(so_path):
        return
    lib = ctypes.CDLL(so_path)
    if not hasattr(lib, "axon_start_nrt_profile"):
        return
    lib.axon_start_nrt_profile.argtypes = [
        ctypes.POINTER(ctypes.c_int64),
        ctypes.c_size_t,
    ]
    lib.axon_start_nrt_profile.restype = ctypes.c_int64
    lib.axon_stop_nrt_profile.argtypes = [ctypes.c_char_p]
    lib.axon_stop_nrt_profile.restype = ctypes.c_int64

    @contextlib.contextmanager
    def _hook(output_dir, device_ids):
        import jax

        jax.devices()
        if device_ids:
            ids = (ctypes.c_int64 * len(device_ids))(*device_ids)
            rc = lib.axon_start_nrt_profile(ids, len(device_ids))
        else:
            rc = lib.axon_start_nrt_profile(None, 0)
        if rc != 0:
            raise RuntimeError(f"axon_start_nrt_profile rc={rc}")
        try:
            yield
        finally:
            n = lib.axon_stop_nrt_profile(str(output_dir).encode())
            if n <= 0:
                print(f"ntff profile: rc={n}, nothing written to {output_dir}")

    mod.set_axon_ntff_profile_hook(_hook)


def kernel(x, W_attn, b_attn, W_proj, b_proj):
    global LAST_RESULT
    nc = build()
    in_maps = make_in_maps(x, W_attn, b_attn, W_proj, b_proj)
    trace = os.environ.get("KERNEL_TRACE", "0") == "1"
    if trace:
        _ensure_ntff_hook()
        import concourse.bass_utils as _bu

        _bu.upload_artifacts = lambda tmpdir: f"local://{tmpdir}"
    res = run_bass_kernel_spmd(
        nc, in_maps, core_ids=list(range(N_CORES)), trace=trace
    )
    LAST_RESULT = res
    outs = [res.results[i]["out"] for i in range(N_CORES)]
    y = np.empty((B, T, C), dtype=np.float32)
    for b in range(B):
        y[b] = outs[4 * b] + outs[4 * b + 1] + outs[4 * b + 2] + outs[4 * b + 3]
    return y


# revision 19
# speedup vs baseline: 1.5873x; 1.0107x over previous
"""Causal self-attention (B=2, T=2048, C=1024, nh=16) on 8 TRN2 NeuronCores.

Sharding: core c -> batch b = c//4, head group g = c%4 (4 heads each).
Each core computes QKV projections for its heads, causal attention, and a
partial output projection (W_proj rows for its heads, + b_proj/4). The four
partials per batch are summed on the host.

Layouts (per core, hardcoded):
  xt   [128, 8, 2048]    x[b].T tiles:  xt[p, kt, t] = x[b, t, kt*128+p]
  wqk  [128, 8, 4, 128]  W_attn q|k cols for this core's heads
  wv   [128, 8, 256]     W_attn v cols
  bqk  [128, 4] f32      b_attn q|k (per-partition bias)
  bv   [1, 256]          b_attn v (added via K=1 matmul)
  wp   [128, 2, 1024]    W_proj rows for this core's heads
  bp4  [1, 1024]         b_proj / 4 (added via K=1 matmul)
  out  [2048, 1024] f32  partial (x[b] @ ... for this head group)

In-kernel dataflow (all matmuls bf16 with fp32 PSUM accumulation):
  qT,kT = (W.T @ x.T)      [feat, t] layout  (lhsT=W tile, rhs=xT)
  v     = (x @ Wv)         [t, feat] layout  (lhsT=xT tile, rhs=Wv)
  S^T   = k @ q.T          [j, i] layout     (lhsT=kT tile, rhs=qT)
  P^T   = exp(S^T/8), masked on diagonal tiles (mult by 0/1 mask)
  y^T,l = [v|1].T @ P^T    [d, i] layout, row 64 = l = sum_j P
  yT    = y^T * (1/l broadcast)
  out   = yT.T @ Wp (+ ones-row * b_proj/4)
"""

import os
import sys

sys.path.insert(0, "/opt/trn_rl_repo")
os.environ.setdefault("MYCRO_LOCAL_CACHE", "1")

import ml_dtypes
import numpy as np

import concourse.bass as bass
import concourse.mybir as mybir
import concourse.tile as tile
from concourse import bacc
from concourse.bass_utils import run_bass_kernel_spmd

B, T, C, NH, HS = 2, 2048, 1024, 16, 64
HPC = 4  # heads per core
N_CORES = 8
KT = C // 128  # 8 contraction tiles over C
TT = T // 128  # 16 tiles over T
IB = T // 512  # 4 i-blocks over T
F32 = mybir.dt.float32

CD = mybir.dt.bfloat16
CD_NP = ml_dtypes.bfloat16

LAST_RESULT = None
_CACHE = {}


def _emit(nc, tc, ctx, aps):
    xt, wqk, wv, bqk, bv, wp, bp4, out = (
        aps["xt"], aps["wqk"], aps["wv"], aps["bqk"], aps["bv"], aps["wp"],
        aps["bp4"], aps["out"],
    )
    Exp = mybir.ActivationFunctionType.Exp
    Identity = mybir.ActivationFunctionType.Identity

    consts = ctx.enter_context(tc.tile_pool(name="consts", bufs=1))

    # --- persistent SBUF tensors ---
    # per-kt tiles + DMAs: fine-grained deps let the first QKV matmuls start
    # as soon as their contraction tile lands instead of waiting for the
    # whole tensor load
    xt_k, wqk_k, wv_k = [], [], []
    for kt in range(KT):
        w = consts.tile([128, 4, 128], CD, tag=f"wqk{kt}")
        nc.sync.dma_start(w[:], wqk[:, kt])
        wqk_k.append(w)
        t = consts.tile([128, T], CD, tag=f"xt{kt}")
        nc.sync.dma_start(t[:], xt[:, kt])
        xt_k.append(t)
        v_ = consts.tile([128, 2, 128], CD, tag=f"wv{kt}")
        nc.sync.dma_start(v_[:], wv[:, kt])
        wv_k.append(v_)
    bqk_s = consts.tile([128, 4], F32, tag="bqk")
    nc.sync.dma_start(bqk_s[:], bqk)
    bv_s = consts.tile([128, 2], F32, tag="bv")
    nc.sync.dma_start(bv_s[:], bv)
    wp_s = consts.tile([128, 2, C], CD, tag="wp")
    nc.sync.dma_start(wp_s[:], wp)
    bp4_s = consts.tile([1, C], CD, tag="bp4")
    nc.sync.dma_start(bp4_s[:], bp4)

    ones1 = consts.tile([1, 128], CD, tag="ones1")
    nc.vector.memset(ones1[:], 1.0)

    # causal mask, shifted-triangle trick: mask[j, c] = 1 if j <= c - 384.
    # diagonal tile with delta = 512*ib - 128*j needs slice [384+delta : +512].
    mask_s = consts.tile([128, 896], CD, tag="mask")
    nc.vector.memset(mask_s[:], 1.0)
    nc.gpsimd.affine_select(
        out=mask_s[:],
        in_=mask_s[:],
        compare_op=mybir.AluOpType.is_ge,
        fill=0.0,
        base=-384,
        channel_multiplier=-1,
        pattern=[[1, 896]],
    )

    qk_t = [consts.tile([128, T], CD, tag=f"q{jt}", name=f"q{jt}")
            for jt in range(2)]
    # kT per head, zero-padded to full 128 partitions: head h occupies rows
    # (h%2)*64..+64, the other 64 rows stay zero. Full-K S-matmuls keep the
    # PE HAM clock-gate warm (K=64 row-masked MMs don't count as PE-busy).
    kz_t = [consts.tile([128, T], CD, tag=f"kz{h}", name=f"kz{h}")
            for h in range(HPC)]
    for h in range(HPC):
        nc.gpsimd.memset(kz_t[h][:], 0.0)
    vT_t = [consts.tile([128, T], CD, tag=f"vT{jt}", name=f"vT{jt}")
            for jt in range(2)]
    vn_t = [consts.tile([128, TT, 128], CD, tag=f"vn{jt}", name=f"vn{jt}")
            for jt in range(2)]
    vext_s = consts.tile([128, TT, HPC * (HS + 1)], CD, tag="vext")
    vext4 = vext_s[:].rearrange("p t (h c) -> p t h c", c=HS + 1)
    nc.gpsimd.memset(vext4[:, :, :, HS], 1.0)  # ones columns
    yt_s = consts.tile([128, 2, T], CD, tag="yt")

    # --- phase 1: q, k, v projections, all W-stationary [feat, t] layout ---
    with tc.tile_pool(name="ph1_psum", bufs=3, space="PSUM") as ph1:
        for jt in range(6):  # 0,1 = q; 2,3 = k; 4,5 = v
            for tb in range(IB):
                ps = ph1.tile([128, 512], F32, tag="qk")
                for kt in range(KT):
                    lhs = (wqk_k[kt][:, jt, :] if jt < 4
                           else wv_k[kt][:, jt - 4, :])
                    nc.tensor.matmul(
                        out=ps[:],
                        lhsT=lhs,
                        rhs=xt_k[kt][:, tb * 512:(tb + 1) * 512],
                        start=(kt == 0),
                        stop=(kt == KT - 1),
                    )
                tsl = slice(tb * 512, (tb + 1) * 512)
                if jt < 2:  # q
                    nc.vector.tensor_scalar_add(
                        qk_t[jt][:, tsl], ps[:], bqk_s[:, jt:jt + 1]
                    )
                elif jt < 4:  # k -> zero-padded per-head kz
                    nc.vector.tensor_scalar_add(
                        kz_t[2 * (jt - 2)][0:64, tsl],
                        ps[0:64, :],
                        bqk_s[0:64, jt:jt + 1],
                    )
                    nc.vector.tensor_scalar_add(
                        kz_t[2 * (jt - 2) + 1][64:128, tsl],
                        ps[64:128, :],
                        bqk_s[64:128, jt:jt + 1],
                    )
                else:  # v -> vT, bias per-partition
                    nc.vector.tensor_scalar_add(
                        vT_t[jt - 4][:, tsl], ps[:], bv_s[:, jt - 4:jt - 3]
                    )
        # vT [feat, t] -> natural [t%128, tt, feat] via DMA transpose, then
        # interleave into vext (65-col per head with the ones column)
        for jt in range(2):
            nc.sync.dma_start_transpose(vn_t[jt][:], vT_t[jt][:])
            nc.vector.tensor_copy(
                vext4[:, :, 2 * jt:2 * jt + 2, 0:HS],
                vn_t[jt][:].rearrange("p t (h c) -> p t h c", c=HS),
            )

    # --- phase 2+3: attention (i-block outer), then that block's projection ---
    # S-tiles are computed two-at-a-time into one 2-bank PSUM tile so a
    # single ACTIVATE(Exp) covers 1024 columns (amortizes ~260ns/op overhead)
    attn_sp = ctx.enter_context(tc.tile_pool(name="attn_s", bufs=2, space="PSUM"))
    attn_yp = ctx.enter_context(tc.tile_pool(name="attn_y", bufs=2, space="PSUM"))
    pt_pool = ctx.enter_context(tc.tile_pool(name="pt", bufs=4))
    misc = ctx.enter_context(tc.tile_pool(name="misc", bufs=4))
    proj_p = ctx.enter_context(tc.tile_pool(name="proj", bufs=2, space="PSUM"))
    stage = ctx.enter_context(tc.tile_pool(name="stage", bufs=3))

    # fixed 128x128 causal triangle (j' <= c') for diagonal strips
    tri = mask_s[:, 384:512]

    for ib in range(IB):
        isl = slice(ib * 512, (ib + 1) * 512)
        for h in range(HPC):
            jt_q = h // 2
            row = (h % 2) * 64
            psy = attn_yp.tile([HS + 1, 512], F32, tag="y")
            njt = 4 * ib + 4
            for j0 in range(0, njt, 2):
                grp = (j0, j0 + 1)
                psS = attn_sp.tile([128, 1024], F32, tag="s")
                pt = pt_pool.tile([128, 2, 512], CD, tag="pt")
                for gi, j in enumerate(grp):
                    o = max(0, 128 * j - 512 * ib)
                    nc.tensor.matmul(
                        out=psS[:, gi * 512: gi * 512 + (512 - o)],
                        lhsT=kz_t[h][:, j * 128:(j + 1) * 128],
                        rhs=qk_t[jt_q][:, ib * 512 + o:(ib + 1) * 512],
                        start=True,
                        stop=True,
                    )
                # one exp per pair when the written PSUM region is contiguous
                # (first tile full); else one exp per written segment
                o0 = max(0, 128 * j0 - 512 * ib)
                o1 = max(0, 128 * (j0 + 1) - 512 * ib)
                ptf = pt[:].rearrange("p n f -> p (n f)")
                if o0 == 0:
                    wflat = 512 + (512 - o1)
                    nc.scalar.activation(
                        out=ptf[:, 0:wflat], in_=psS[:, 0:wflat],
                        func=Exp, scale=0.125,
                    )
                else:
                    nc.scalar.activation(
                        out=ptf[:, 0:512 - o0], in_=psS[:, 0:512 - o0],
                        func=Exp, scale=0.125,
                    )
                    nc.scalar.activation(
                        out=ptf[:, 512:1024 - o1], in_=psS[:, 512:1024 - o1],
                        func=Exp, scale=0.125,
                    )
                for gi, j in enumerate(grp):
                    o = max(0, 128 * j - 512 * ib)
                    if 128 * j >= 512 * ib:  # diagonal tile -> mask boundary
                        nc.vector.tensor_mul(
                            pt[:, gi, 0:128], pt[:, gi, 0:128], tri
                        )
                    nc.tensor.matmul(
                        out=psy[:, o:512],
                        lhsT=vext4[:, j, h, :],
                        rhs=pt[:, gi, 0:512 - o],
                        start=(j == 0),
                        stop=(j == njt - 1),
                    )
            lrow = misc.tile([1, 512], F32, tag="lrow")
            nc.vector.tensor_copy(lrow[:], psy[HS:HS + 1, :])
            lbc = misc.tile([64, 512], F32, tag="lbc")
            nc.gpsimd.partition_broadcast(lbc[:], lrow[:], channels=64)
            linv = misc.tile([64, 512], F32, tag="linv")
            nc.vector.reciprocal_approx_fast(linv[:], lbc[:])
            nc.vector.tensor_mul(
                yt_s[row:row + 64, jt_q, isl], psy[0:HS, :], linv[:]
            )

        for tloc in range(4):
            ttp = ib * 4 + tloc
            for eb in range(2):
                psp = proj_p.tile([128, 512], F32, tag="p")
                for dt in range(2):
                    nc.tensor.matmul(
                        out=psp[:],
                        lhsT=yt_s[:, dt, ttp * 128:(ttp + 1) * 128],
                        rhs=wp_s[:, dt, eb * 512:(eb + 1) * 512],
                        start=(dt == 0),
                        stop=False,
                    )
                nc.tensor.matmul(
                    out=psp[:],
                    lhsT=ones1[:],
                    rhs=bp4_s[:, eb * 512:(eb + 1) * 512],
                    start=False,
                    stop=True,
                )
                st = stage.tile([128, 512], F32, tag="st")
                nc.vector.tensor_copy(st[:], psp[:])
                nc.sync.dma_start(
                    out[ttp * 128:(ttp + 1) * 128, eb * 512:(eb + 1) * 512], st[:]
                )


def build():
    if "nc" in _CACHE:
        return _CACHE["nc"]
    nc = bacc.Bacc(
        "TRN2", target_bir_lowering=False, debug=False, num_devices=N_CORES
    )
    aps = {
        "xt": nc.dram_tensor("xt", [128, KT, T], CD, kind="ExternalInput").ap(),
        "wqk": nc.dram_tensor("wqk", [128, KT, 4, 128], CD, kind="ExternalInput").ap(),
        "wv": nc.dram_tensor("wv", [128, KT, 2, 128], CD, kind="ExternalInput").ap(),
        "bqk": nc.dram_tensor("bqk", [128, 4], F32, kind="ExternalInput").ap(),
        "bv": nc.dram_tensor("bv", [128, 2], F32, kind="ExternalInput").ap(),
        "wp": nc.dram_tensor("wp", [128, 2, C], CD, kind="ExternalInput").ap(),
        "bp4": nc.dram_tensor("bp4", [1, C], CD, kind="ExternalInput").ap(),
        "out": nc.dram_tensor("out", [T, C], F32, kind="ExternalOutput").ap(),
    }
    from contextlib import ExitStack

    with tile.TileContext(nc) as tc:
        with ExitStack() as ctx:
            _emit(nc, tc, ctx, aps)
    nc.compile()
    _CACHE["nc"] = nc
    return nc


def make_in_maps(x, W_attn, b_attn, W_proj, b_proj):
    x = np.asarray(x, dtype=np.float32)
    W_attn = np.asarray(W_attn, dtype=np.float32)
    b_attn = np.asarray(b_attn, dtype=np.float32)
    W_proj = np.asarray(W_proj, dtype=np.float32)
    b_proj = np.asarray(b_proj, dtype=np.float32)

    in_maps = []
    xt_b = {}
    for b in range(B):
        xt = np.ascontiguousarray(x[b].T)  # [C, T]
        xt_b[b] = (
            xt.reshape(KT, 128, T).transpose(1, 0, 2).astype(CD_NP)
        )
    for core in range(N_CORES):
        b = core // 4
        g = core % 4
        fs = slice(256 * g, 256 * g + 256)  # feature cols for this head group
        wq = W_attn[:, fs]
        wk = W_attn[:, C + 256 * g: C + 256 * g + 256]
        wv = W_attn[:, 2 * C + 256 * g: 2 * C + 256 * g + 256]
        wqk = np.concatenate([wq, wk], axis=1)  # [1024, 512]
        bq = b_attn[fs]
        bk = b_attn[C + 256 * g: C + 256 * g + 256]
        bv = b_attn[2 * C + 256 * g: 2 * C + 256 * g + 256]
        in_maps.append({
            "xt": xt_b[b],
            "wqk": np.ascontiguousarray(
                wqk.reshape(KT, 128, 4, 128).transpose(1, 0, 2, 3)
            ).astype(CD_NP),
            "wv": np.ascontiguousarray(
                wv.reshape(KT, 128, 2, 128).transpose(1, 0, 2, 3)
            ).astype(CD_NP),
            "bqk": np.ascontiguousarray(
                np.concatenate([bq, bk]).reshape(4, 128).T
            ).astype(np.float32),
            "bv": np.ascontiguousarray(bv.reshape(2, 128).T).astype(np.float32),
            "wp": np.ascontiguousarray(
                W_proj[fs, :].reshape(2, 128, C).transpose(1, 0, 2)
            ).astype(CD_NP),
            "bp4": (b_proj / 4)[None, :].astype(CD_NP),
        })
    return in_maps


def _ensure_ntff_hook():
    """Recreate the missing antenv.axon_hooks NTFF-profile shim (see
    trn_agent_boot/trn_boot.py) so run_bass_kernel_spmd(trace=True) works."""
    import contextlib
    import ctypes
    import types

    try:
        from antenv.axon_hooks import get_axon_ntff_profile_hook  # noqa: F401

        return
    except ImportError:
        pass

    mod = types.ModuleType("antenv.axon_hooks")
    _holder = {"hook": None}
    mod.set_axon_ntff_profile_hook = lambda h: _holder.__setitem__("hook", h)
    mod.get_axon_ntff_profile_hook = lambda: _holder["hook"]
    sys.modules["antenv.axon_hooks"] = mod
    import antenv

    antenv.axon_hooks = mod

    so_path = "/opt/axon/libaxon_pjrt.so"
    if not os.path.exists(so_path):
        return
    lib = ctypes.CDLL(so_path)
    if not hasattr(lib, "axon_start_nrt_profile"):
        return
    lib.axon_start_nrt_profile.argtypes = [
        ctypes.POINTER(ctypes.c_int64),
        ctypes.c_size_t,
    ]
    lib.axon_start_nrt_profile.restype = ctypes.c_int64
    lib.axon_stop_nrt_profile.argtypes = [ctypes.c_char_p]
    lib.axon_stop_nrt_profile.restype = ctypes.c_int64

    @contextlib.contextmanager
    def _hook(output_dir, device_ids):
        import jax

        jax.devices()
        if device_ids:
            ids = (ctypes.c_int64 * len(device_ids))(*device_ids)
            rc = lib.axon_start_nrt_profile(ids, len(device_ids))
        else:
            rc = lib.axon_start_nrt_profile(None, 0)
        if rc != 0:
            raise RuntimeError(f"axon_start_nrt_profile rc={rc}")
        try:
            yield
        finally:
            n = lib.axon_stop_nrt_profile(str(output_dir).encode())
            if n <= 0:
                print(f"ntff profile: rc={n}, nothing written to {output_dir}")

    mod.set_axon_ntff_profile_hook(_hook)


def kernel(x, W_attn, b_attn, W_proj, b_proj):
    global LAST_RESULT
    nc = build()
    in_maps = make_in_maps(x, W_attn, b_attn, W_proj, b_proj)
    trace = os.environ.get("KERNEL_TRACE", "0") == "1"
    if trace:
        _ensure_ntff_hook()
        import concourse.bass_utils as _bu

        _bu.upload_artifacts = lambda tmpdir: f"local://{tmpdir}"
    res = run_bass_kernel_spmd(
        nc, in_maps, core_ids=list(range(N_CORES)), trace=trace
    )
    LAST_RESULT = res
    outs = [res.results[i]["out"] for i in range(N_CORES)]
    y = np.empty((B, T, C), dtype=np.float32)
    for b in range(B):
        y[b] = outs[4 * b] + outs[4 * b + 1] + outs[4 * b + 2] + outs[4 * b + 3]
    return y
